# revision 14
# baseline (speedup 1.0000x reference)
"""EpisodicMemory retrieval kernel for 8 Trainium2 NeuronCores.

Sharding (hardcoded for the nn_EpisodicMemory problem):
  - q = buffer_states.reshape(-1) [25600]: contraction-sharded for layer 1
    (each core gets q[3200i:3200(i+1)] and W1 rows [3200i:3200(i+1), :]),
    partial pre-activations summed with an on-device AllReduce.
  - W2 replicated; W3 column-sharded (core i computes enc[512i:512(i+1)]),
    assembled with an on-device AllGather.
  - episodes_encoded row-sharded: core i scores episodes [1250i:1250(i+1)),
    computes local top-3, decodes them locally with a replicated Wd1/Wd2.
  - host merges the 8x3 candidates into the global top-3 and averages the
    matching decoded vectors (pure gather/selection glue).

Activation vectors h1/h2 are kept in feature-major [128, nch] layout
(partition p, chunk c == feature 128*c+p) so PE matmuls need no transposes;
LayerNorm stats cross partitions via ACT accumulate + a ones-vector matmul.
"""

import numpy as np

DIM = 256
WIN = 100
COMP = 16
NEP = 10000
NCORES = 8

Q = WIN * DIM            # 25600
H1 = 4 * DIM             # 1024
H2 = 2 * DIM             # 512
E = COMP * DIM           # 4096
QS = Q // NCORES         # 3200 rows of W1 per core
ESLC = E // NCORES       # 512 enc columns per core
ES = NEP // NCORES       # 1250 episodes per core
EPT = 10                 # episode tiles per core
EPP = ES // EPT          # 125 partitions used per episode tile
K = 3
EPS = 1e-5

_compiled = None


def build_kernel(gelu_func_name: str = "Gelu", stage: int = 99):
    import concourse.bacc as bacc
    import concourse.bass as bass
    import concourse.tile as tile
    import concourse.mybir as mybir

    f32 = mybir.dt.float32
    u32 = mybir.dt.uint32
    bf16 = mybir.dt.bfloat16
    AF = mybir.ActivationFunctionType
    GELU = getattr(AF, gelu_func_name)
    OP = mybir.AluOpType

    nc = bacc.Bacc("TRN2", target_bir_lowering=False, debug=False,
                   enable_asserts=True, num_devices=NCORES)

    # ---- I/O ----
    q_s = nc.dram_tensor("q_s", [QS], f32, kind="ExternalInput").ap()
    W1_s = nc.dram_tensor("W1_s", [QS, H1], f32, kind="ExternalInput").ap()
    W2 = nc.dram_tensor("W2", [H1, H2], f32, kind="ExternalInput").ap()
    W3_s = nc.dram_tensor("W3_s", [H2, ESLC], f32, kind="ExternalInput").ap()
    ep_s = nc.dram_tensor("ep_s", [ES, E], f32, kind="ExternalInput").ap()
    Wd1 = nc.dram_tensor("Wd1", [E, H2], f32, kind="ExternalInput").ap()
    Wd2 = nc.dram_tensor("Wd2", [H2, DIM], f32, kind="ExternalInput").ap()
    b1v = nc.dram_tensor("b1v", [H1], f32, kind="ExternalInput").ap()
    g1v = nc.dram_tensor("g1v", [H1], f32, kind="ExternalInput").ap()
    be1v = nc.dram_tensor("be1v", [H1], f32, kind="ExternalInput").ap()
    b2v = nc.dram_tensor("b2v", [H2], f32, kind="ExternalInput").ap()
    g2v = nc.dram_tensor("g2v", [H2], f32, kind="ExternalInput").ap()
    be2v = nc.dram_tensor("be2v", [H2], f32, kind="ExternalInput").ap()
    b3s = nc.dram_tensor("b3s", [1, ESLC], f32, kind="ExternalInput").ap()
    bd1v = nc.dram_tensor("bd1v", [1, H2], f32, kind="ExternalInput").ap()
    gdv = nc.dram_tensor("gdv", [1, H2], f32, kind="ExternalInput").ap()
    bedv = nc.dram_tensor("bedv", [1, H2], f32, kind="ExternalInput").ap()
    bd2v = nc.dram_tensor("bd2v", [1, DIM], f32, kind="ExternalInput").ap()
    eye3 = nc.dram_tensor("eye3", [3, 3], f32, kind="ExternalInput").ap()

    loc_out = nc.dram_tensor("loc_out", [K, DIM], f32, kind="ExternalOutput").ap()
    loc_sims = nc.dram_tensor("loc_sims", [1, 8], f32, kind="ExternalOutput").ap()

    W1v = W1_s.rearrange("(kc p) n -> kc p n", p=128)          # [25,128,1024]
    W3v = W3_s.rearrange("(kc p) n -> kc p n", p=128)          # [4,128,512]
    epv = ep_s.rearrange("(p t) d -> t p d", t=EPT)            # [10,125,4096]
    Wd1v = Wd1.rearrange("(kc p) n -> kc p n", p=128)          # [32,128,512]

    C1 = H1 // 128   # 8 feature chunks of h1
    C2 = H2 // 128   # 4 feature chunks of h2

    with tile.TileContext(nc) as tc:
        with tc.tile_pool(name="dram", bufs=1, space="DRAM") as dram, \
             tc.tile_pool(name="const", bufs=1) as const, \
             tc.tile_pool(name="w1p", bufs=2) as w1p, \
             tc.tile_pool(name="encp", bufs=1) as encp, \
             tc.tile_pool(name="epp", bufs=4) as eppool, \
             tc.tile_pool(name="trash", bufs=1) as trashp, \
             tc.tile_pool(name="wd1p", bufs=2) as wd1p, \
             tc.tile_pool(name="small", bufs=1) as small, \
             tc.tile_pool(name="psum", bufs=1, space="PSUM") as psum, \
             tc.tile_pool(name="psum_kc", bufs=2, space="PSUM") as psum_kc, \
             tc.tile_pool(name="psum_tp", bufs=2, space="PSUM") as psum_tp:

            # ---------- constants into SBUF ----------
            qsb = const.tile([128, QS // 128], f32, tag="qsb")          # [128,25]
            nc.sync.dma_start(out=qsb[:, :], in_=q_s.rearrange("(kc p) -> p kc", p=128))
            W2sb = const.tile([128, C1, H2], f32, tag="w2sb")           # [128,8,512]
            nc.sync.dma_start(out=W2sb[:, :, :], in_=W2.rearrange("(kc p) n -> p kc n", p=128))
            Wd2sb = const.tile([128, C2, DIM], f32, tag="wd2sb")        # [128,4,256]
            nc.sync.dma_start(out=Wd2sb[:, :, :], in_=Wd2.rearrange("(kc p) n -> p kc n", p=128))

            # feature-major bias/gain vectors [128, nch]
            b1m = const.tile([128, C1], f32, tag="b1m")
            nc.sync.dma_start(out=b1m[:, :], in_=b1v.rearrange("(kc p) -> p kc", p=128))
            g1m = const.tile([128, C1], f32, tag="g1m")
            nc.sync.dma_start(out=g1m[:, :], in_=g1v.rearrange("(kc p) -> p kc", p=128))
            be1m = const.tile([128, C1], f32, tag="be1m")
            nc.sync.dma_start(out=be1m[:, :], in_=be1v.rearrange("(kc p) -> p kc", p=128))
            b2m = const.tile([128, C2], f32, tag="b2m")
            nc.sync.dma_start(out=b2m[:, :], in_=b2v.rearrange("(kc p) -> p kc", p=128))
            g2m = const.tile([128, C2], f32, tag="g2m")
            nc.sync.dma_start(out=g2m[:, :], in_=g2v.rearrange("(kc p) -> p kc", p=128))
            be2m = const.tile([128, C2], f32, tag="be2m")
            nc.sync.dma_start(out=be2m[:, :], in_=be2v.rearrange("(kc p) -> p kc", p=128))

            b3ssb = const.tile([1, ESLC], f32, tag="b3ssb")
            nc.sync.dma_start(out=b3ssb[:, :], in_=b3s[:, :])
            bd1sb = const.tile([K, H2], f32, tag="bd1sb")
            nc.sync.dma_start(out=bd1sb[:, :], in_=bd1v.to_broadcast([K, H2]))
            gdsb = const.tile([K, H2], f32, tag="gdsb")
            nc.sync.dma_start(out=gdsb[:, :], in_=gdv.to_broadcast([K, H2]))
            bedsb = const.tile([K, H2], f32, tag="bedsb")
            nc.sync.dma_start(out=bedsb[:, :], in_=bedv.to_broadcast([K, H2]))
            bd2sb = const.tile([K, DIM], f32, tag="bd2sb")
            nc.sync.dma_start(out=bd2sb[:, :], in_=bd2v.to_broadcast([K, DIM]))
            eye3sb = const.tile([3, 3], f32, tag="eye3sb")
            nc.sync.dma_start(out=eye3sb[:, :], in_=eye3[:, :])
            eps1 = const.tile([1, 1], f32, tag="eps1")
            nc.vector.memset(eps1[:, :], EPS)
            eps3 = const.tile([K, 1], f32, tag="eps3")
            nc.vector.memset(eps3[:, :], EPS)
            ones128 = const.tile([128, 1], f32, tag="ones128")
            nc.vector.memset(ones128[:, :], 1.0)

            # DRAM bounce buffers
            ar1_in = dram.tile([128, C1], f32)
            ar1_out = dram.tile([128, C1], f32)
            ag3_in = dram.tile([1, ESLC], f32)
            ag3_out = dram.tile([1, E], f32)
            flat_d = dram.tile([ES], f32)
            idx_d = dram.tile([K], u32)

            def stage1_e1():
                """partial h1_pre = q_s @ W1_s  -> [128, 8], AllReduce."""
                h1pre = small.tile([128, C1], f32, tag="h1pre")
                nc.vector.memset(h1pre[:, :], 0.0)
                nkc = QS // 128  # 25
                for kc in range(nkc):
                    w1t = w1p.tile([128, H1], f32, tag="w1")
                    nc.sync.dma_start(out=w1t[:, :], in_=W1v[kc])
                    pkc = psum_kc.tile([128, C1], f32, tag="pkc")
                    for mc in range(C1):
                        nc.tensor.matmul(
                            out=pkc[:, mc:mc + 1],
                            lhsT=w1t[:, 128 * mc:128 * (mc + 1)],
                            rhs=qsb[:, kc:kc + 1],
                            start=True, stop=True,
                        )
                    nc.vector.tensor_add(out=h1pre[:, :], in0=h1pre[:, :], in1=pkc[:, :])
                nc.sync.dma_start(out=ar1_in[:, :], in_=h1pre[:, :])
                nc.gpsimd.collective_compute(
                    "AllReduce", OP.add,
                    replica_groups=[list(range(NCORES))],
                    ins=[ar1_in.opt()], outs=[ar1_out.opt()],
                )
                return h1pre

            def ln_feature_major(xm, nch, nfeat, bm, gm, bem, name):
                """gelu+LN on feature-major [128, nch] tile xm (in-place)."""
                nc.vector.tensor_add(out=xm[:, :], in0=xm[:, :], in1=bm[:, :])
                s12 = small.tile([128, 2], f32, tag=f"s12_{name}")
                nc.scalar.activation(out=xm[:, :], in_=xm[:, :], func=GELU,
                                     accum_out=s12[:, 0:1])
                t8 = small.tile([128, nch], f32, tag=f"t8_{name}")
                nc.scalar.activation(out=t8[:, :], in_=xm[:, :], func=AF.Square,
                                     accum_out=s12[:, 1:2])
                sp = psum.tile([1, 2], f32, tag="sp_ln")
                nc.tensor.matmul(out=sp[:, :], lhsT=ones128[:, :], rhs=s12[:, :],
                                 start=True, stop=True)
                mu = small.tile([1, 2], f32, tag=f"mu_{name}")
                nc.scalar.activation(out=mu[:, :], in_=sp[:, :], func=AF.Copy,
                                     scale=1.0 / nfeat)   # [mean, E[x^2]]
                msq = small.tile([1, 1], f32, tag=f"msq_{name}")
                nc.scalar.activation(out=msq[:, :], in_=mu[:, 0:1], func=AF.Square)
                var = small.tile([1, 1], f32, tag=f"var_{name}")
                nc.vector.tensor_sub(out=var[:, :], in0=mu[:, 1:2], in1=msq[:, :])
                nc.scalar.activation(out=var[:, :], in_=var[:, :], func=AF.Sqrt,
                                     bias=eps1[:, :])
                nc.vector.reciprocal(out=var[:, :], in_=var[:, :])
                pk = small.tile([1, 2], f32, tag=f"pk_{name}")
                nc.vector.tensor_copy(out=pk[:, 0:1], in_=mu[:, 0:1])
                nc.vector.tensor_copy(out=pk[:, 1:2], in_=var[:, :])
                pkb = small.tile([128, 2], f32, tag=f"pkb_{name}")
                nc.gpsimd.partition_broadcast(pkb[:, :], pk[:, :])
                nc.vector.tensor_scalar(
                    out=xm[:, :], in0=xm[:, :],
                    scalar1=pkb[:, 0:1], scalar2=pkb[:, 1:2],
                    op0=OP.subtract, op1=OP.mult,
                )
                nc.vector.tensor_mul(out=xm[:, :], in0=xm[:, :], in1=gm[:, :])
                nc.vector.tensor_add(out=xm[:, :], in0=xm[:, :], in1=bem[:, :])

            def stage2_encoder():
                """E1 epilogue + E2 + E3 + AllGather -> encb [128, E]."""
                h1 = small.tile([128, C1], f32, tag="h1")
                nc.sync.dma_start(out=h1[:, :], in_=ar1_out[:, :])
                ln_feature_major(h1, C1, H1, b1m, g1m, be1m, "l1")

                h2 = small.tile([128, C2], f32, tag="h2")
                nc.vector.memset(h2[:, :], 0.0)
                for kc in range(C1):
                    pkc = psum_kc.tile([128, C2], f32, tag="pkc")
                    for mc in range(C2):
                        nc.tensor.matmul(
                            out=pkc[:, mc:mc + 1],
                            lhsT=W2sb[:, kc, 128 * mc:128 * (mc + 1)],
                            rhs=h1[:, kc:kc + 1],
                            start=True, stop=True,
                        )
                    nc.vector.tensor_add(out=h2[:, :], in0=h2[:, :], in1=pkc[:, :C2])
                ln_feature_major(h2, C2, H2, b2m, g2m, be2m, "l2")

                e3p = psum.tile([1, ESLC], f32, tag="e3p")
                for kc in range(C2):
                    w3t = wd1p.tile([128, ESLC], f32, tag="wd1")
                    nc.sync.dma_start(out=w3t[:, :], in_=W3v[kc])
                    nc.tensor.matmul(
                        out=e3p[:, :], lhsT=h2[:, kc:kc + 1], rhs=w3t[:, :],
                        start=(kc == 0), stop=(kc == C2 - 1),
                    )
                encsl = small.tile([1, ESLC], f32, tag="encsl")
                nc.vector.tensor_copy(out=encsl[:, :], in_=e3p[:, :])
                nc.vector.tensor_add(out=encsl[:, :], in0=encsl[:, :], in1=b3ssb[:, :])
                nc.sync.dma_start(out=ag3_in[:, :], in_=encsl[:, :])
                nc.gpsimd.collective_compute(
                    "AllGather", OP.bypass,
                    replica_groups=[list(range(NCORES))],
                    ins=[ag3_in.opt()], outs=[ag3_out.opt()],
                )
                encb = encp.tile([128, E], f32, tag="encb")
                nc.sync.dma_start(out=encb[:, :], in_=ag3_out.to_broadcast([128, E]))
                return encb

            def stage3_sims(encb):
                """episode dots + norms + normalize + local top-8."""
                sraw = small.tile([128, EPT], f32, tag="sraw")
                nsq = small.tile([128, EPT], f32, tag="nsq")
                trash = trashp.tile([EPP, E], bf16, tag="trash")
                trash2 = trashp.tile([EPP, E], bf16, tag="trash2")
                for t in range(EPT):
                    et = eppool.tile([EPP, E], f32, tag="ep")
                    nc.sync.dma_start(out=et[:, :], in_=epv[t])
                    nc.scalar.activation(out=trash[:, :], in_=et[:, :], func=AF.Square,
                                         accum_out=nsq[:EPP, t:t + 1])
                    nc.vector.tensor_tensor(out=trash2[:, :], in0=et[:, :],
                                            in1=encb[:EPP, :], op=OP.mult)
                    nc.scalar.activation(out=trash2[:, :], in_=trash2[:, :],
                                         func=AF.Copy,
                                         accum_out=sraw[:EPP, t:t + 1])

                nstd = small.tile([128, EPT], f32, tag="nstd")
                nc.scalar.activation(out=nstd[:EPP, :], in_=nsq[:EPP, :], func=AF.Sqrt)
                nc.vector.reciprocal(out=nstd[:EPP, :], in_=nstd[:EPP, :])
                snorm = small.tile([128, EPT], f32, tag="snorm")
                nc.vector.tensor_mul(out=snorm[:EPP, :], in0=sraw[:EPP, :],
                                     in1=nstd[:EPP, :])
                # [125,10] -> flat [1,1250] via DRAM (flat index == local episode id)
                nc.sync.dma_start(out=flat_d.rearrange("(p t) -> p t", t=EPT),
                                  in_=snorm[:EPP, :])
                flat = small.tile([1, ES], f32, tag="flat")
                nc.sync.dma_start(out=flat[:1, :],
                                  in_=flat_d.rearrange("(a n) -> a n", a=1))
                vals = small.tile([1, 8], f32, tag="vals")
                idx8 = small.tile([1, 8], u32, tag="idx8")
                if stage >= 25:
                    nc.vector.max(out=vals[:, :], in_=flat[:, :])
                    nc.vector.max_index(out=idx8[:, :], in_max=vals[:, :],
                                        in_values=flat[:, :])
                else:
                    nc.vector.tensor_copy(out=vals[:, :], in_=flat[:, :8])
                    nc.vector.memset(idx8[:, :], 0)
                return vals, idx8

            def stage4_gather(idx8):
                """top-3 indices -> per-partition; indirect gather rows [3, E]."""
                nc.sync.dma_start(out=idx_d.rearrange("(a n) -> a n", a=1),
                                  in_=idx8[:, 0:K])
                idx3 = small.tile([K, 1], u32, tag="idx3")
                nc.sync.dma_start(out=idx3[:, :],
                                  in_=idx_d.rearrange("(p o) -> p o", o=1))
                rows = small.tile([K, E], f32, tag="rows")
                nc.gpsimd.indirect_dma_start(
                    out=rows[:, :], out_offset=None,
                    in_=ep_s[:, :],
                    in_offset=bass.IndirectOffsetOnAxis(ap=idx3[:, :1], axis=0),
                )
                return rows

            def stage5_decoder(rows):
                rowsT = small.tile([128, E // 128, K], f32, tag="rowsT")
                for kc in range(E // 128):
                    tp = psum_tp.tile([128, K], f32, tag="tp")
                    nc.tensor.transpose(out=tp[:, :],
                                        in_=rows[:, 128 * kc:128 * (kc + 1)],
                                        identity=eye3sb[:, :])
                    nc.vector.tensor_copy(out=rowsT[:, kc, :], in_=tp[:, :])
                pdp = psum.tile([K, H2], f32, tag="pdp")
                for kc in range(E // 128):
                    wt = wd1p.tile([128, H2], f32, tag="wd1")
                    nc.sync.dma_start(out=wt[:, :], in_=Wd1v[kc])
                    nc.tensor.matmul(
                        out=pdp[:, :], lhsT=rowsT[:, kc, :], rhs=wt[:, :],
                        start=(kc == 0), stop=(kc == E // 128 - 1),
                    )
                d = small.tile([K, H2], f32, tag="d")
                nc.vector.tensor_copy(out=d[:, :], in_=pdp[:, :])
                nc.vector.tensor_add(out=d[:, :], in0=d[:, :], in1=bd1sb[:, :])
                nc.scalar.activation(out=d[:, :], in_=d[:, :], func=GELU)
                std = small.tile([K, 6], f32, tag="std")
                nc.vector.bn_stats(out=std[:, :], in_=d[:, :])
                mvd = small.tile([K, 2], f32, tag="mvd")
                nc.vector.bn_aggr(out=mvd[:, :], in_=std[:, :])
                rstdd = small.tile([K, 1], f32, tag="rstdd")
                nc.scalar.activation(out=rstdd[:, :], in_=mvd[:, 1:2], func=AF.Sqrt,
                                     bias=eps3[:, :])
                nc.vector.reciprocal(out=rstdd[:, :], in_=rstdd[:, :])
                nc.vector.tensor_scalar(
                    out=d[:, :], in0=d[:, :],
                    scalar1=mvd[:, 0:1], scalar2=rstdd[:, :],
                    op0=OP.subtract, op1=OP.mult,
                )
                nc.vector.tensor_mul(out=d[:, :], in0=d[:, :], in1=gdsb[:, :])
                nc.vector.tensor_add(out=d[:, :], in0=d[:, :], in1=bedsb[:, :])

                dT = small.tile([128, C2, K], f32, tag="dT")
                for kc in range(C2):
                    tp = psum_tp.tile([128, K], f32, tag="tp")
                    nc.tensor.transpose(out=tp[:, :],
                                        in_=d[:, 128 * kc:128 * (kc + 1)],
                                        identity=eye3sb[:, :])
                    nc.vector.tensor_copy(out=dT[:, kc, :], in_=tp[:, :])
                o3p = psum.tile([K, DIM], f32, tag="o3p")
                for kc in range(C2):
                    nc.tensor.matmul(
                        out=o3p[:, :], lhsT=dT[:, kc, :], rhs=Wd2sb[:, kc, :],
                        start=(kc == 0), stop=(kc == C2 - 1),
                    )
                o3 = small.tile([K, DIM], f32, tag="o3")
                nc.vector.tensor_copy(out=o3[:, :], in_=o3p[:, :])
                nc.vector.tensor_add(out=o3[:, :], in0=o3[:, :], in1=bd2sb[:, :])
                nc.sync.dma_start(out=loc_out[:, :], in_=o3[:, :])

            h1pre = stage1_e1()
            if stage <= 1:
                nc.sync.dma_start(out=loc_sims[:, :], in_=h1pre[:1, :8])
            else:
                encb = stage2_encoder()
                if stage <= 2:
                    nc.sync.dma_start(out=loc_sims[:, :], in_=encb[:1, :8])
                else:
                    vals, idx8 = stage3_sims(encb)  # stages 21,22,23,24,25
                    nc.sync.dma_start(out=loc_sims[:, :], in_=vals[:, :])
                    if stage > 25:
                        rows = stage4_gather(idx8)
                        if stage <= 26:
                            nc.sync.dma_start(out=loc_out[:1, :], in_=rows[:1, :DIM])
                        else:
                            stage5_decoder(rows)

    nc.compile()
    return nc


def _shard_inputs(buffer_states, episodes_encoded, W1, b1, g1, be1, W2, b2, g2,
                  be2, W3, b3, Wd1, bd1, gd, bed, Wd2, bd2):
    q = np.ascontiguousarray(buffer_states, dtype=np.float32).reshape(-1)
    eye3 = np.eye(3, dtype=np.float32)
    in_maps = []
    for i in range(NCORES):
        m = {
            "q_s": np.ascontiguousarray(q[QS * i:QS * (i + 1)]),
            "W1_s": np.ascontiguousarray(W1[QS * i:QS * (i + 1)]),
            "W2": W2,
            "W3_s": np.ascontiguousarray(W3[:, ESLC * i:ESLC * (i + 1)]),
            "ep_s": np.ascontiguousarray(episodes_encoded[ES * i:ES * (i + 1)]),
            "Wd1": Wd1,
            "Wd2": Wd2,
            "b1v": b1, "g1v": g1, "be1v": be1,
            "b2v": b2, "g2v": g2, "be2v": be2,
            "b3s": np.ascontiguousarray(b3.reshape(1, -1)[:, ESLC * i:ESLC * (i + 1)]),
            "bd1v": bd1.reshape(1, -1), "gdv": gd.reshape(1, -1),
            "bedv": bed.reshape(1, -1), "bd2v": bd2.reshape(1, -1),
            "eye3": eye3,
        }
        in_maps.append(m)
    return in_maps


def _merge(results):
    sims24 = np.concatenate([r["loc_sims"][0, :K] for r in results])     # [24]
    outs24 = np.concatenate([r["loc_out"] for r in results], axis=0)     # [24, 256]
    top = np.argsort(-sims24, kind="stable")[:K]
    return outs24[top].mean(axis=0).astype(np.float32)


def kernel(*, trace=False, **inputs):
    global _compiled
    from concourse.bass_utils import run_bass_kernel_spmd

    k = int(inputs.pop("k"))
    assert k == K, f"kernel hardcodes k=3, got {k}"
    arrs = {name: np.ascontiguousarray(np.asarray(v, dtype=np.float32))
            for name, v in inputs.items()}
    in_maps = _shard_inputs(
        arrs["buffer_states"], arrs["episodes_encoded"],
        arrs["W1"], arrs["b1"], arrs["g1"], arrs["be1"],
        arrs["W2"], arrs["b2"], arrs["g2"], arrs["be2"],
        arrs["W3"], arrs["b3"], arrs["Wd1"], arrs["bd1"], arrs["gd"],
        arrs["bed"], arrs["Wd2"], arrs["bd2"],
    )
    if _compiled is None:
        _compiled = build_kernel()
    res = run_bass_kernel_spmd(_compiled, in_maps, core_ids=list(range(NCORES)),
                               trace=trace)
    out = _merge(res.results)
    if trace:
        kernel.last_exec_time_ns = res.exec_time_ns
    return out


kernel.last_exec_time_ns = None


# revision 15
# speedup vs baseline: 1.3357x; 1.3357x over previous
"""EpisodicMemory retrieval kernel for 8 Trainium2 NeuronCores.

Sharding (hardcoded for the nn_EpisodicMemory problem):
  - q = buffer_states.reshape(-1) [25600]: contraction-sharded for layer 1
    (each core gets q[3200i:3200(i+1)] and W1 rows [3200i:3200(i+1), :]),
    partial pre-activations summed with an on-device AllReduce.
  - W2 replicated; W3 column-sharded (core i computes enc[512i:512(i+1)]),
    assembled with an on-device AllGather.
  - episodes_encoded row-sharded: core i scores episodes [1250i:1250(i+1)),
    computes local top-3, decodes them locally with a replicated Wd1/Wd2.
  - host merges the 8x3 candidates into the global top-3 and averages the
    matching decoded vectors (pure gather/selection glue).

Precision: weights are cast to bf16 on the host. The encoder only influences
WHICH episodes are selected (top-3 margins are ~10%), the decoder matmuls
accumulate in fp32 PSUM, and episode data stays fp32, so output error stays
small. Set BF16=False to fall back to full fp32.
"""

import numpy as np

DIM = 256
WIN = 100
COMP = 16
NEP = 10000
NCORES = 8

Q = WIN * DIM            # 25600
H1 = 4 * DIM             # 1024
H2 = 2 * DIM             # 512
E = COMP * DIM           # 4096
QS = Q // NCORES         # 3200 rows of W1 per core
ESLC = E // NCORES       # 512 enc columns per core
ES = NEP // NCORES       # 1250 episodes per core
EPT = 10                 # episode tiles per core
EPP = ES // EPT          # 125 partitions used per episode tile
K = 3
EPS = 1e-5
BF16 = True
EP_BUFS = 5

_compiled = None


def build_kernel(gelu_func_name: str = "Gelu"):
    import concourse.bacc as bacc
    import concourse.bass as bass
    import concourse.tile as tile
    import concourse.mybir as mybir

    f32 = mybir.dt.float32
    u32 = mybir.dt.uint32
    bf16 = mybir.dt.bfloat16
    wdt = bf16 if BF16 else f32
    AF = mybir.ActivationFunctionType
    GELU = getattr(AF, gelu_func_name)
    OP = mybir.AluOpType

    nc = bacc.Bacc("TRN2", target_bir_lowering=False, debug=False,
                   enable_asserts=True, num_devices=NCORES)

    # ---- I/O ----
    q_s = nc.dram_tensor("q_s", [QS], wdt, kind="ExternalInput").ap()
    W1_s = nc.dram_tensor("W1_s", [QS, H1], wdt, kind="ExternalInput").ap()
    W2 = nc.dram_tensor("W2", [H1, H2], wdt, kind="ExternalInput").ap()
    W3_s = nc.dram_tensor("W3_s", [H2, ESLC], wdt, kind="ExternalInput").ap()
    ep_s = nc.dram_tensor("ep_s", [ES, E], f32, kind="ExternalInput").ap()
    Wd1 = nc.dram_tensor("Wd1", [E, H2], wdt, kind="ExternalInput").ap()
    Wd2 = nc.dram_tensor("Wd2", [H2, DIM], wdt, kind="ExternalInput").ap()
    b1v = nc.dram_tensor("b1v", [H1], f32, kind="ExternalInput").ap()
    g1v = nc.dram_tensor("g1v", [H1], f32, kind="ExternalInput").ap()
    be1v = nc.dram_tensor("be1v", [H1], f32, kind="ExternalInput").ap()
    b2v = nc.dram_tensor("b2v", [H2], f32, kind="ExternalInput").ap()
    g2v = nc.dram_tensor("g2v", [H2], f32, kind="ExternalInput").ap()
    be2v = nc.dram_tensor("be2v", [H2], f32, kind="ExternalInput").ap()
    b3s = nc.dram_tensor("b3s", [1, ESLC], f32, kind="ExternalInput").ap()
    bd1v = nc.dram_tensor("bd1v", [1, H2], f32, kind="ExternalInput").ap()
    gdv = nc.dram_tensor("gdv", [1, H2], f32, kind="ExternalInput").ap()
    bedv = nc.dram_tensor("bedv", [1, H2], f32, kind="ExternalInput").ap()
    bd2v = nc.dram_tensor("bd2v", [1, DIM], f32, kind="ExternalInput").ap()
    eye3 = nc.dram_tensor("eye3", [3, 3], f32, kind="ExternalInput").ap()

    loc_out = nc.dram_tensor("loc_out", [K, DIM], f32, kind="ExternalOutput").ap()
    loc_sims = nc.dram_tensor("loc_sims", [1, 8], f32, kind="ExternalOutput").ap()

    W1v = W1_s.rearrange("(kc p) n -> kc p n", p=128)          # [25,128,1024]
    W2v = W2.rearrange("(kc p) n -> kc p n", p=128)            # [8,128,512]
    W3v = W3_s.rearrange("(kc p) n -> kc p n", p=128)          # [4,128,512]
    epv = ep_s.rearrange("(p t) d -> t p d", t=EPT)            # [10,125,4096]
    Wd1v = Wd1.rearrange("(kc p) n -> kc p n", p=128)          # [32,128,512]

    C1 = H1 // 128   # 8
    C2 = H2 // 128   # 4
    EH = E // 2      # 2048 split point for the dot reduce

    with tile.TileContext(nc) as tc:
        with tc.tile_pool(name="dram", bufs=1, space="DRAM") as dram, \
             tc.tile_pool(name="const", bufs=1) as const, \
             tc.tile_pool(name="w1p", bufs=4) as w1p, \
             tc.tile_pool(name="encp", bufs=1) as encp, \
             tc.tile_pool(name="epp", bufs=EP_BUFS) as eppool, \
             tc.tile_pool(name="trash", bufs=1) as trashp, \
             tc.tile_pool(name="wd1p", bufs=4) as wd1p, \
             tc.tile_pool(name="small", bufs=1) as small, \
             tc.tile_pool(name="psum", bufs=1, space="PSUM") as psum, \
             tc.tile_pool(name="psum_tp", bufs=2, space="PSUM") as psum_tp:

            # ---------- constants ----------
            qsb = const.tile([128, QS // 128], wdt, tag="qsb")
            nc.sync.dma_start(out=qsb[:, :], in_=q_s.rearrange("(kc p) -> p kc", p=128))
            Wd2sb = const.tile([128, C2, DIM], wdt, tag="wd2sb")
            nc.sync.dma_start(out=Wd2sb[:, :, :], in_=Wd2.rearrange("(kc p) n -> p kc n", p=128))

            b1sb = const.tile([1, H1], f32, tag="b1sb")
            nc.sync.dma_start(out=b1sb[:, :], in_=b1v.rearrange("(a n) -> a n", a=1))
            g1sb = const.tile([1, H1], f32, tag="g1sb")
            nc.sync.dma_start(out=g1sb[:, :], in_=g1v.rearrange("(a n) -> a n", a=1))
            be1sb = const.tile([1, H1], f32, tag="be1sb")
            nc.sync.dma_start(out=be1sb[:, :], in_=be1v.rearrange("(a n) -> a n", a=1))
            b2sb = const.tile([1, H2], f32, tag="b2sb")
            nc.sync.dma_start(out=b2sb[:, :], in_=b2v.rearrange("(a n) -> a n", a=1))
            g2sb = const.tile([1, H2], f32, tag="g2sb")
            nc.sync.dma_start(out=g2sb[:, :], in_=g2v.rearrange("(a n) -> a n", a=1))
            be2sb = const.tile([1, H2], f32, tag="be2sb")
            nc.sync.dma_start(out=be2sb[:, :], in_=be2v.rearrange("(a n) -> a n", a=1))
            b3ssb = const.tile([1, ESLC], f32, tag="b3ssb")
            nc.sync.dma_start(out=b3ssb[:, :], in_=b3s[:, :])
            bd1sb = const.tile([K, H2], f32, tag="bd1sb")
            nc.sync.dma_start(out=bd1sb[:, :], in_=bd1v.to_broadcast([K, H2]))
            gdsb = const.tile([K, H2], f32, tag="gdsb")
            nc.sync.dma_start(out=gdsb[:, :], in_=gdv.to_broadcast([K, H2]))
            bedsb = const.tile([K, H2], f32, tag="bedsb")
            nc.sync.dma_start(out=bedsb[:, :], in_=bedv.to_broadcast([K, H2]))
            bd2sb = const.tile([K, DIM], f32, tag="bd2sb")
            nc.sync.dma_start(out=bd2sb[:, :], in_=bd2v.to_broadcast([K, DIM]))
            eye3sb = const.tile([3, 3], f32, tag="eye3sb")
            nc.sync.dma_start(out=eye3sb[:, :], in_=eye3[:, :])
            eps1 = const.tile([1, 1], f32, tag="eps1")
            nc.vector.memset(eps1[:, :], EPS)
            eps3 = const.tile([K, 1], f32, tag="eps3")
            nc.vector.memset(eps3[:, :], EPS)

            # DRAM bounce/scratch
            ar1_in = dram.tile([H1], f32)
            ar1_out = dram.tile([H1], f32)
            ag3_in = dram.tile([1, ESLC], f32)
            ag3_out = dram.tile([1, E], f32)
            h1_d = dram.tile([H1], wdt)
            h2_d = dram.tile([H2], wdt)
            flat_d = dram.tile([ES], f32)
            idx_d = dram.tile([K], u32)

            # ======== E1: h1_pre = q_s @ W1_s  -> psum [1, 1024] ========
            e1p = psum.tile([1, H1], f32, tag="e1p")
            nkc = QS // 128  # 25
            for kc in range(nkc):
                w1t = w1p.tile([128, H1], wdt, tag="w1")
                nc.sync.dma_start(out=w1t[:, :], in_=W1v[kc])
                for h in range(2):
                    nc.tensor.matmul(
                        out=e1p[:, 512 * h:512 * (h + 1)],
                        lhsT=qsb[:, kc:kc + 1],
                        rhs=w1t[:, 512 * h:512 * (h + 1)],
                        start=(kc == 0), stop=(kc == nkc - 1),
                    )
            h1f = small.tile([1, H1], f32, tag="h1flat")
            nc.vector.tensor_copy(out=h1f[:, :], in_=e1p[:, :])
            nc.sync.dma_start(out=ar1_in.rearrange("(a n) -> a n", a=1), in_=h1f[:, :])
            nc.gpsimd.collective_compute(
                "AllReduce", OP.add,
                replica_groups=[list(range(NCORES))],
                ins=[ar1_in.opt()], outs=[ar1_out.opt()],
            )

            def ln_flat(xf, width, bsb, gsb, besb, name):
                """+bias, gelu, LN on a [1, width] tile, in place."""
                nc.vector.tensor_add(out=xf[:, :], in0=xf[:, :], in1=bsb[:, :])
                nc.scalar.activation(out=xf[:, :], in_=xf[:, :], func=GELU)
                nsub = (width + 511) // 512
                st = small.tile([1, nsub, 6], f32, tag=f"st_{name}")
                for sg in range(nsub):
                    nc.vector.bn_stats(out=st[:, sg, :],
                                       in_=xf[:, 512 * sg:512 * (sg + 1)])
                mv = small.tile([1, 2], f32, tag=f"mv_{name}")
                nc.vector.bn_aggr(out=mv[:, :], in_=st[:, :, :])
                rstd = small.tile([1, 1], f32, tag=f"rstd_{name}")
                nc.scalar.activation(out=rstd[:, :], in_=mv[:, 1:2], func=AF.Sqrt,
                                     bias=eps1[:, :])
                nc.vector.reciprocal(out=rstd[:, :], in_=rstd[:, :])
                nc.vector.tensor_scalar(
                    out=xf[:, :], in0=xf[:, :],
                    scalar1=mv[:, 0:1], scalar2=rstd[:, :],
                    op0=OP.subtract, op1=OP.mult,
                )
                nc.vector.tensor_mul(out=xf[:, :], in0=xf[:, :], in1=gsb[:, :])
                nc.vector.tensor_add(out=xf[:, :], in0=xf[:, :], in1=besb[:, :])

            # ---------- E1 epilogue: flat LN then to feature-major bf16 ----------
            h1 = small.tile([1, H1], f32, tag="h1flat")
            nc.sync.dma_start(out=h1[:, :], in_=ar1_out.rearrange("(a n) -> a n", a=1))
            ln_flat(h1, H1, b1sb, g1sb, be1sb, "l1")
            h1c = small.tile([1, H1], wdt, tag="h1c")
            nc.vector.tensor_copy(out=h1c[:, :], in_=h1[:, :])
            nc.sync.dma_start(out=h1_d.rearrange("(a n) -> a n", a=1), in_=h1c[:, :])
            h1m = small.tile([128, C1], wdt, tag="h1m")
            nc.sync.dma_start(out=h1m[:, :], in_=h1_d.rearrange("(kc p) -> p kc", p=128))

            # ======== E2 ========
            e23p = psum.tile([1, H2], f32, tag="e23p")
            for kc in range(C1):
                w2t = w1p.tile([128, H2], wdt, tag="w1")
                nc.sync.dma_start(out=w2t[:, :], in_=W2v[kc])
                nc.tensor.matmul(
                    out=e23p[:, :], lhsT=h1m[:, kc:kc + 1], rhs=w2t[:, :],
                    start=(kc == 0), stop=(kc == C1 - 1),
                )
            h2 = small.tile([1, H2], f32, tag="h2flat")
            nc.vector.tensor_copy(out=h2[:, :], in_=e23p[:, :])
            ln_flat(h2, H2, b2sb, g2sb, be2sb, "l2")
            h2c = small.tile([1, H2], wdt, tag="h2c")
            nc.vector.tensor_copy(out=h2c[:, :], in_=h2[:, :])
            nc.sync.dma_start(out=h2_d.rearrange("(a n) -> a n", a=1), in_=h2c[:, :])
            h2m = small.tile([128, C2], wdt, tag="h2m")
            nc.sync.dma_start(out=h2m[:, :], in_=h2_d.rearrange("(kc p) -> p kc", p=128))

            # ======== E3 ========
            e3p = psum.tile([1, ESLC], f32, tag="e23p")
            for kc in range(C2):
                w3t = w1p.tile([128, ESLC], wdt, tag="w1")
                nc.sync.dma_start(out=w3t[:, :], in_=W3v[kc])
                nc.tensor.matmul(
                    out=e3p[:, :], lhsT=h2m[:, kc:kc + 1], rhs=w3t[:, :],
                    start=(kc == 0), stop=(kc == C2 - 1),
                )
            encsl = small.tile([1, ESLC], f32, tag="encsl")
            nc.vector.tensor_copy(out=encsl[:, :], in_=e3p[:, :])
            nc.vector.tensor_add(out=encsl[:, :], in0=encsl[:, :], in1=b3ssb[:, :])
            nc.sync.dma_start(out=ag3_in[:, :], in_=encsl[:, :])
            nc.gpsimd.collective_compute(
                "AllGather", OP.bypass,
                replica_groups=[list(range(NCORES))],
                ins=[ag3_in.opt()], outs=[ag3_out.opt()],
            )
            encb = encp.tile([128, E], f32, tag="encb")
            nc.sync.dma_start(out=encb[:, :], in_=ag3_out.to_broadcast([128, E]))

            # ======== episodes: norms (ACT) + dots (DVE mult, split reduce) ====
            dotA = small.tile([128, EPT], f32, tag="dotA")
            dotB = small.tile([128, EPT], f32, tag="dotB")
            nsq = small.tile([128, EPT], f32, tag="nsq")
            trash = trashp.tile([EPP, E], bf16, tag="trash")
            trash2 = trashp.tile([EPP, E], bf16, tag="trash2")
            for t in range(EPT):
                et = eppool.tile([EPP, E], f32, tag="ep")
                nc.sync.dma_start(out=et[:, :], in_=epv[t])
                nc.scalar.activation(out=trash[:, :], in_=et[:, :], func=AF.Square,
                                     accum_out=nsq[:EPP, t:t + 1])
                nc.vector.tensor_tensor(out=trash2[:, :], in0=et[:, :],
                                        in1=encb[:EPP, :], op=OP.mult)
                nc.scalar.activation(out=trash2[:, :EH], in_=trash2[:, :EH],
                                     func=AF.Copy, accum_out=dotA[:EPP, t:t + 1])
                nc.vector.tensor_reduce(out=dotB[:EPP, t:t + 1],
                                        in_=trash2[:, EH:],
                                        axis=mybir.AxisListType.X, op=OP.add)

            # ======== normalize + local top-k ========
            sraw = small.tile([128, EPT], f32, tag="sraw")
            nc.vector.tensor_add(out=sraw[:EPP, :], in0=dotA[:EPP, :], in1=dotB[:EPP, :])
            nstd = small.tile([128, EPT], f32, tag="nstd")
            nc.scalar.activation(out=nstd[:EPP, :], in_=nsq[:EPP, :], func=AF.Sqrt)
            nc.vector.reciprocal(out=nstd[:EPP, :], in_=nstd[:EPP, :])
            snorm = small.tile([128, EPT], f32, tag="snorm")
            nc.vector.tensor_mul(out=snorm[:EPP, :], in0=sraw[:EPP, :], in1=nstd[:EPP, :])
            nc.sync.dma_start(out=flat_d.rearrange("(p t) -> p t", t=EPT),
                              in_=snorm[:EPP, :])
            flat = small.tile([1, ES], f32, tag="flat")
            nc.sync.dma_start(out=flat[:1, :],
                              in_=flat_d.rearrange("(a n) -> a n", a=1))
            vals = small.tile([1, 8], f32, tag="vals")
            nc.vector.max(out=vals[:, :], in_=flat[:, :])
            idx8 = small.tile([1, 8], u32, tag="idx8")
            nc.vector.max_index(out=idx8[:, :], in_max=vals[:, :], in_values=flat[:, :])
            nc.sync.dma_start(out=idx_d.rearrange("(a n) -> a n", a=1),
                              in_=idx8[:, 0:K])
            idx3 = small.tile([K, 1], u32, tag="idx3")
            nc.sync.dma_start(out=idx3[:, :],
                              in_=idx_d.rearrange("(p o) -> p o", o=1))

            rows = small.tile([K, E], f32, tag="rows")
            nc.gpsimd.indirect_dma_start(
                out=rows[:, :], out_offset=None,
                in_=ep_s[:, :],
                in_offset=bass.IndirectOffsetOnAxis(ap=idx3[:, :1], axis=0),
            )

            # ======== decoder ========
            rowsT = small.tile([128, E // 128, K], wdt, tag="rowsT")
            for kc in range(E // 128):
                tp = psum_tp.tile([128, K], f32, tag="tp")
                nc.tensor.transpose(out=tp[:, :], in_=rows[:, 128 * kc:128 * (kc + 1)],
                                    identity=eye3sb[:, :])
                nc.vector.tensor_copy(out=rowsT[:, kc, :], in_=tp[:, :])
            pdp = psum.tile([K, H2], f32, tag="pdp")
            for kc in range(E // 128):
                wt = wd1p.tile([128, H2], wdt, tag="wd1")
                nc.sync.dma_start(out=wt[:, :], in_=Wd1v[kc])
                nc.tensor.matmul(
                    out=pdp[:, :], lhsT=rowsT[:, kc, :], rhs=wt[:, :],
                    start=(kc == 0), stop=(kc == E // 128 - 1),
                )
            d = small.tile([K, H2], f32, tag="d")
            nc.vector.tensor_copy(out=d[:, :], in_=pdp[:, :])
            nc.vector.tensor_add(out=d[:, :], in0=d[:, :], in1=bd1sb[:, :])
            nc.scalar.activation(out=d[:, :], in_=d[:, :], func=GELU)
            std = small.tile([K, 6], f32, tag="std")
            nc.vector.bn_stats(out=std[:, :], in_=d[:, :])
            mvd = small.tile([K, 2], f32, tag="mvd")
            nc.vector.bn_aggr(out=mvd[:, :], in_=std[:, :])
            rstdd = small.tile([K, 1], f32, tag="rstdd")
            nc.scalar.activation(out=rstdd[:, :], in_=mvd[:, 1:2], func=AF.Sqrt,
                                 bias=eps3[:, :])
            nc.vector.reciprocal(out=rstdd[:, :], in_=rstdd[:, :])
            nc.vector.tensor_scalar(
                out=d[:, :], in0=d[:, :],
                scalar1=mvd[:, 0:1], scalar2=rstdd[:, :],
                op0=OP.subtract, op1=OP.mult,
            )
            nc.vector.tensor_mul(out=d[:, :], in0=d[:, :], in1=gdsb[:, :])
            nc.vector.tensor_add(out=d[:, :], in0=d[:, :], in1=bedsb[:, :])

            dT = small.tile([128, C2, K], wdt, tag="dT")
            for kc in range(C2):
                tp = psum_tp.tile([128, K], f32, tag="tp")
                nc.tensor.transpose(out=tp[:, :], in_=d[:, 128 * kc:128 * (kc + 1)],
                                    identity=eye3sb[:, :])
                nc.vector.tensor_copy(out=dT[:, kc, :], in_=tp[:, :])
            o3p = psum.tile([K, DIM], f32, tag="o3p")
            for kc in range(C2):
                nc.tensor.matmul(
                    out=o3p[:, :], lhsT=dT[:, kc, :], rhs=Wd2sb[:, kc, :],
                    start=(kc == 0), stop=(kc == C2 - 1),
                )
            o3 = small.tile([K, DIM], f32, tag="o3")
            nc.vector.tensor_copy(out=o3[:, :], in_=o3p[:, :])
            nc.vector.tensor_add(out=o3[:, :], in0=o3[:, :], in1=bd2sb[:, :])

            nc.sync.dma_start(out=loc_out[:, :], in_=o3[:, :])
            nc.sync.dma_start(out=loc_sims[:, :], in_=vals[:, :])

    nc.compile()
    return nc


def _wcast(a):
    if not BF16:
        return np.ascontiguousarray(a, dtype=np.float32)
    import ml_dtypes
    return np.ascontiguousarray(np.asarray(a, dtype=np.float32).astype(ml_dtypes.bfloat16))


def _shard_inputs(buffer_states, episodes_encoded, W1, b1, g1, be1, W2, b2, g2,
                  be2, W3, b3, Wd1, bd1, gd, bed, Wd2, bd2):
    q = np.ascontiguousarray(buffer_states, dtype=np.float32).reshape(-1)
    eye3 = np.eye(3, dtype=np.float32)
    W2c = _wcast(W2)
    Wd1c = _wcast(Wd1)
    Wd2c = _wcast(Wd2)
    in_maps = []
    for i in range(NCORES):
        m = {
            "q_s": _wcast(q[QS * i:QS * (i + 1)]),
            "W1_s": _wcast(W1[QS * i:QS * (i + 1)]),
            "W2": W2c,
            "W3_s": _wcast(W3[:, ESLC * i:ESLC * (i + 1)]),
            "ep_s": np.ascontiguousarray(episodes_encoded[ES * i:ES * (i + 1)]),
            "Wd1": Wd1c,
            "Wd2": Wd2c,
            "b1v": b1, "g1v": g1, "be1v": be1,
            "b2v": b2, "g2v": g2, "be2v": be2,
            "b3s": np.ascontiguousarray(b3.reshape(1, -1)[:, ESLC * i:ESLC * (i + 1)]),
            "bd1v": bd1.reshape(1, -1), "gdv": gd.reshape(1, -1),
            "bedv": bed.reshape(1, -1), "bd2v": bd2.reshape(1, -1),
            "eye3": eye3,
        }
        in_maps.append(m)
    return in_maps


def _merge(results):
    sims24 = np.concatenate([r["loc_sims"][0, :K] for r in results])     # [24]
    outs24 = np.concatenate([r["loc_out"] for r in results], axis=0)     # [24, 256]
    top = np.argsort(-sims24, kind="stable")[:K]
    return outs24[top].mean(axis=0).astype(np.float32)


def kernel(*, trace=False, **inputs):
    global _compiled
    from concourse.bass_utils import run_bass_kernel_spmd

    k = int(inputs.pop("k"))
    assert k == K, f"kernel hardcodes k=3, got {k}"
    arrs = {name: np.ascontiguousarray(np.asarray(v, dtype=np.float32))
            for name, v in inputs.items()}
    in_maps = _shard_inputs(
        arrs["buffer_states"], arrs["episodes_encoded"],
        arrs["W1"], arrs["b1"], arrs["g1"], arrs["be1"],
        arrs["W2"], arrs["b2"], arrs["g2"], arrs["be2"],
        arrs["W3"], arrs["b3"], arrs["Wd1"], arrs["bd1"], arrs["gd"],
        arrs["bed"], arrs["Wd2"], arrs["bd2"],
    )
    if _compiled is None:
        _compiled = build_kernel()
    res = run_bass_kernel_spmd(_compiled, in_maps, core_ids=list(range(NCORES)),
                               trace=trace)
    out = _merge(res.results)
    if trace:
        kernel.last_exec_time_ns = res.exec_time_ns
    return out


kernel.last_exec_time_ns = None


# revision 16
# speedup vs baseline: 1.4315x; 1.0717x over previous
"""EpisodicMemory retrieval kernel for 8 Trainium2 NeuronCores.

Sharding (hardcoded for the nn_EpisodicMemory problem):
  - q = buffer_states.reshape(-1) [25600]: contraction-sharded for layer 1
    (each core gets q[3200i:3200(i+1)] and W1 rows [3200i:3200(i+1), :]),
    partial pre-activations summed with an on-device AllReduce.
  - W2 replicated; W3 column-sharded (core i computes enc[512i:512(i+1)]),
    assembled with an on-device AllGather.
  - episodes_encoded row-sharded: core i scores episodes [1250i:1250(i+1)),
    computes local top-3, decodes them locally with a replicated Wd1/Wd2.
  - host merges the 8x3 candidates into the global top-3 and averages the
    matching decoded vectors (pure gather/selection glue).

Precision: weights are cast to bf16 on the host. The encoder only influences
WHICH episodes are selected (top-3 margins are ~10%), the decoder matmuls
accumulate in fp32 PSUM, and episode data stays fp32, so output error stays
small. Set BF16=False to fall back to full fp32.
"""

import numpy as np

DIM = 256
WIN = 100
COMP = 16
NEP = 10000
NCORES = 8

Q = WIN * DIM            # 25600
H1 = 4 * DIM             # 1024
H2 = 2 * DIM             # 512
E = COMP * DIM           # 4096
QS = Q // NCORES         # 3200 rows of W1 per core
ESLC = E // NCORES       # 512 enc columns per core
ES = NEP // NCORES       # 1250 episodes per core
EPT = 10                 # episode tiles per core
EPP = ES // EPT          # 125 partitions used per episode tile
K = 3
EPS = 1e-5
BF16 = True
EP_BUFS = 5

_compiled = None


def build_kernel(gelu_func_name: str = "Gelu"):
    import concourse.bacc as bacc
    import concourse.bass as bass
    import concourse.tile as tile
    import concourse.mybir as mybir
    from concourse.tile import add_dep_helper

    f32 = mybir.dt.float32
    u32 = mybir.dt.uint32
    bf16 = mybir.dt.bfloat16
    wdt = bf16 if BF16 else f32
    AF = mybir.ActivationFunctionType
    GELU = getattr(AF, gelu_func_name)
    OP = mybir.AluOpType

    nc = bacc.Bacc("TRN2", target_bir_lowering=False, debug=False,
                   enable_asserts=True, num_devices=NCORES)

    # ---- I/O ----
    q_s = nc.dram_tensor("q_s", [QS], wdt, kind="ExternalInput").ap()
    W1_s = nc.dram_tensor("W1_s", [QS, H1], wdt, kind="ExternalInput").ap()
    W2 = nc.dram_tensor("W2", [H1, H2], wdt, kind="ExternalInput").ap()
    W3_s = nc.dram_tensor("W3_s", [H2, ESLC], wdt, kind="ExternalInput").ap()
    ep_s = nc.dram_tensor("ep_s", [ES, E], f32, kind="ExternalInput").ap()
    Wd1 = nc.dram_tensor("Wd1", [E, H2], wdt, kind="ExternalInput").ap()
    Wd2 = nc.dram_tensor("Wd2", [H2, DIM], wdt, kind="ExternalInput").ap()
    b1v = nc.dram_tensor("b1v", [H1], f32, kind="ExternalInput").ap()
    g1v = nc.dram_tensor("g1v", [H1], f32, kind="ExternalInput").ap()
    be1v = nc.dram_tensor("be1v", [H1], f32, kind="ExternalInput").ap()
    b2v = nc.dram_tensor("b2v", [H2], f32, kind="ExternalInput").ap()
    g2v = nc.dram_tensor("g2v", [H2], f32, kind="ExternalInput").ap()
    be2v = nc.dram_tensor("be2v", [H2], f32, kind="ExternalInput").ap()
    b3s = nc.dram_tensor("b3s", [1, ESLC], f32, kind="ExternalInput").ap()
    bd1v = nc.dram_tensor("bd1v", [1, H2], f32, kind="ExternalInput").ap()
    gdv = nc.dram_tensor("gdv", [1, H2], f32, kind="ExternalInput").ap()
    bedv = nc.dram_tensor("bedv", [1, H2], f32, kind="ExternalInput").ap()
    bd2v = nc.dram_tensor("bd2v", [1, DIM], f32, kind="ExternalInput").ap()
    eye3 = nc.dram_tensor("eye3", [3, 3], f32, kind="ExternalInput").ap()

    loc_out = nc.dram_tensor("loc_out", [K, DIM], f32, kind="ExternalOutput").ap()
    loc_sims = nc.dram_tensor("loc_sims", [1, 8], f32, kind="ExternalOutput").ap()

    W1v = W1_s.rearrange("(kc p) n -> kc p n", p=128)          # [25,128,1024]
    W2v = W2.rearrange("(kc p) n -> kc p n", p=128)            # [8,128,512]
    W3v = W3_s.rearrange("(kc p) n -> kc p n", p=128)          # [4,128,512]
    epv = ep_s.rearrange("(p t) d -> t p d", t=EPT)            # [10,125,4096]
    Wd1v = Wd1.rearrange("(kc p) n -> kc p n", p=128)          # [32,128,512]

    C1 = H1 // 128   # 8
    C2 = H2 // 128   # 4
    EH = E // 2      # 2048 split point for the dot reduce

    with tile.TileContext(nc) as tc:
        with tc.tile_pool(name="dram", bufs=1, space="DRAM") as dram, \
             tc.tile_pool(name="const", bufs=1) as const, \
             tc.tile_pool(name="w1p", bufs=4) as w1p, \
             tc.tile_pool(name="encp", bufs=1) as encp, \
             tc.tile_pool(name="epp", bufs=EP_BUFS) as eppool, \
             tc.tile_pool(name="trash", bufs=1) as trashp, \
             tc.tile_pool(name="trash2", bufs=2) as trash2p, \
             tc.tile_pool(name="wd1p", bufs=4) as wd1p, \
             tc.tile_pool(name="small", bufs=1) as small, \
             tc.tile_pool(name="psum", bufs=1, space="PSUM") as psum, \
             tc.tile_pool(name="psum_tp", bufs=2, space="PSUM") as psum_tp:

            # ---------- constants ----------
            qsb = const.tile([128, QS // 128], wdt, tag="qsb")
            nc.sync.dma_start(out=qsb[:, :], in_=q_s.rearrange("(kc p) -> p kc", p=128))
            Wd2sb = const.tile([128, C2, DIM], wdt, tag="wd2sb")
            wd2_dma = nc.sync.dma_start(out=Wd2sb[:, :, :], in_=Wd2.rearrange("(kc p) n -> p kc n", p=128))

            b1sb = const.tile([1, H1], f32, tag="b1sb")
            nc.sync.dma_start(out=b1sb[:, :], in_=b1v.rearrange("(a n) -> a n", a=1))
            g1sb = const.tile([1, H1], f32, tag="g1sb")
            nc.sync.dma_start(out=g1sb[:, :], in_=g1v.rearrange("(a n) -> a n", a=1))
            be1sb = const.tile([1, H1], f32, tag="be1sb")
            nc.sync.dma_start(out=be1sb[:, :], in_=be1v.rearrange("(a n) -> a n", a=1))
            late_dmas = []
            b2sb = const.tile([1, H2], f32, tag="b2sb")
            late_dmas.append(nc.sync.dma_start(out=b2sb[:, :], in_=b2v.rearrange("(a n) -> a n", a=1)))
            g2sb = const.tile([1, H2], f32, tag="g2sb")
            late_dmas.append(nc.sync.dma_start(out=g2sb[:, :], in_=g2v.rearrange("(a n) -> a n", a=1)))
            be2sb = const.tile([1, H2], f32, tag="be2sb")
            late_dmas.append(nc.sync.dma_start(out=be2sb[:, :], in_=be2v.rearrange("(a n) -> a n", a=1)))
            b3ssb = const.tile([1, ESLC], f32, tag="b3ssb")
            late_dmas.append(nc.sync.dma_start(out=b3ssb[:, :], in_=b3s[:, :]))
            bd1sb = const.tile([K, H2], f32, tag="bd1sb")
            late_dmas.append(nc.sync.dma_start(out=bd1sb[:, :], in_=bd1v.to_broadcast([K, H2])))
            gdsb = const.tile([K, H2], f32, tag="gdsb")
            late_dmas.append(nc.sync.dma_start(out=gdsb[:, :], in_=gdv.to_broadcast([K, H2])))
            bedsb = const.tile([K, H2], f32, tag="bedsb")
            late_dmas.append(nc.sync.dma_start(out=bedsb[:, :], in_=bedv.to_broadcast([K, H2])))
            bd2sb = const.tile([K, DIM], f32, tag="bd2sb")
            late_dmas.append(nc.sync.dma_start(out=bd2sb[:, :], in_=bd2v.to_broadcast([K, DIM])))
            eye3sb = const.tile([3, 3], f32, tag="eye3sb")
            late_dmas.append(nc.sync.dma_start(out=eye3sb[:, :], in_=eye3[:, :]))
            eps1 = const.tile([1, 1], f32, tag="eps1")
            nc.vector.memset(eps1[:, :], EPS)
            eps3 = const.tile([K, 1], f32, tag="eps3")
            nc.vector.memset(eps3[:, :], EPS)

            # DRAM bounce/scratch
            ar1_in = dram.tile([H1], f32)
            ar1_out = dram.tile([H1], f32)
            ag3_in = dram.tile([1, ESLC], f32)
            ag3_out = dram.tile([1, E], f32)
            h1_d = dram.tile([H1], wdt)
            h2_d = dram.tile([H2], wdt)
            flat_d = dram.tile([ES], f32)
            idx_d = dram.tile([K], u32)

            # ======== E1: h1_pre = q_s @ W1_s  -> psum [1, 1024] ========
            e1p = psum.tile([1, H1], f32, tag="e1p")
            nkc = QS // 128  # 25
            for kc in range(nkc):
                w1t = w1p.tile([128, H1], wdt, tag="w1")
                nc.sync.dma_start(out=w1t[:, :], in_=W1v[kc])
                for h in range(2):
                    nc.tensor.matmul(
                        out=e1p[:, 512 * h:512 * (h + 1)],
                        lhsT=qsb[:, kc:kc + 1],
                        rhs=w1t[:, 512 * h:512 * (h + 1)],
                        start=(kc == 0), stop=(kc == nkc - 1),
                    )
            h1f = small.tile([1, H1], f32, tag="h1flat")
            nc.vector.tensor_copy(out=h1f[:, :], in_=e1p[:, :])
            ar1_write = nc.sync.dma_start(out=ar1_in.rearrange("(a n) -> a n", a=1), in_=h1f[:, :])
            for _h in late_dmas + [wd2_dma]:
                add_dep_helper(_h.ins, ar1_write.ins, reason="defer const load past E1 stream")
            nc.gpsimd.collective_compute(
                "AllReduce", OP.add,
                replica_groups=[list(range(NCORES))],
                ins=[ar1_in.opt()], outs=[ar1_out.opt()],
            )

            def ln_flat(xf, width, bsb, gsb, besb, name):
                """+bias, gelu, LN on a [1, width] tile, in place."""
                nc.vector.tensor_add(out=xf[:, :], in0=xf[:, :], in1=bsb[:, :])
                nc.scalar.activation(out=xf[:, :], in_=xf[:, :], func=GELU)
                nsub = (width + 511) // 512
                st = small.tile([1, nsub, 6], f32, tag=f"st_{name}")
                for sg in range(nsub):
                    nc.vector.bn_stats(out=st[:, sg, :],
                                       in_=xf[:, 512 * sg:512 * (sg + 1)])
                mv = small.tile([1, 2], f32, tag=f"mv_{name}")
                nc.vector.bn_aggr(out=mv[:, :], in_=st[:, :, :])
                rstd = small.tile([1, 1], f32, tag=f"rstd_{name}")
                nc.scalar.activation(out=rstd[:, :], in_=mv[:, 1:2], func=AF.Sqrt,
                                     bias=eps1[:, :])
                nc.vector.reciprocal(out=rstd[:, :], in_=rstd[:, :])
                nc.vector.tensor_scalar(
                    out=xf[:, :], in0=xf[:, :],
                    scalar1=mv[:, 0:1], scalar2=rstd[:, :],
                    op0=OP.subtract, op1=OP.mult,
                )
                nc.vector.tensor_mul(out=xf[:, :], in0=xf[:, :], in1=gsb[:, :])
                nc.vector.tensor_add(out=xf[:, :], in0=xf[:, :], in1=besb[:, :])

            # ---------- E1 epilogue: flat LN then to feature-major bf16 ----------
            h1 = small.tile([1, H1], f32, tag="h1flat")
            nc.sync.dma_start(out=h1[:, :], in_=ar1_out.rearrange("(a n) -> a n", a=1))
            ln_flat(h1, H1, b1sb, g1sb, be1sb, "l1")
            h1c = small.tile([1, H1], wdt, tag="h1c")
            nc.vector.tensor_copy(out=h1c[:, :], in_=h1[:, :])
            nc.sync.dma_start(out=h1_d.rearrange("(a n) -> a n", a=1), in_=h1c[:, :])
            h1m = small.tile([128, C1], wdt, tag="h1m")
            nc.sync.dma_start(out=h1m[:, :], in_=h1_d.rearrange("(kc p) -> p kc", p=128))

            # ======== E2 ========
            e23p = psum.tile([1, H2], f32, tag="e23p")
            for kc in range(C1):
                w2t = w1p.tile([128, H2], wdt, tag="w1")
                nc.sync.dma_start(out=w2t[:, :], in_=W2v[kc])
                nc.tensor.matmul(
                    out=e23p[:, :], lhsT=h1m[:, kc:kc + 1], rhs=w2t[:, :],
                    start=(kc == 0), stop=(kc == C1 - 1),
                )
            h2 = small.tile([1, H2], f32, tag="h2flat")
            nc.vector.tensor_copy(out=h2[:, :], in_=e23p[:, :])
            ln_flat(h2, H2, b2sb, g2sb, be2sb, "l2")
            h2c = small.tile([1, H2], wdt, tag="h2c")
            nc.vector.tensor_copy(out=h2c[:, :], in_=h2[:, :])
            nc.sync.dma_start(out=h2_d.rearrange("(a n) -> a n", a=1), in_=h2c[:, :])
            h2m = small.tile([128, C2], wdt, tag="h2m")
            nc.sync.dma_start(out=h2m[:, :], in_=h2_d.rearrange("(kc p) -> p kc", p=128))

            # ======== E3 ========
            e3p = psum.tile([1, ESLC], f32, tag="e23p")
            for kc in range(C2):
                w3t = w1p.tile([128, ESLC], wdt, tag="w1")
                nc.sync.dma_start(out=w3t[:, :], in_=W3v[kc])
                nc.tensor.matmul(
                    out=e3p[:, :], lhsT=h2m[:, kc:kc + 1], rhs=w3t[:, :],
                    start=(kc == 0), stop=(kc == C2 - 1),
                )
            encsl = small.tile([1, ESLC], f32, tag="encsl")
            nc.vector.tensor_copy(out=encsl[:, :], in_=e3p[:, :])
            nc.vector.tensor_add(out=encsl[:, :], in0=encsl[:, :], in1=b3ssb[:, :])
            nc.sync.dma_start(out=ag3_in[:, :], in_=encsl[:, :])
            nc.gpsimd.collective_compute(
                "AllGather", OP.bypass,
                replica_groups=[list(range(NCORES))],
                ins=[ag3_in.opt()], outs=[ag3_out.opt()],
            )
            encb = encp.tile([128, E], f32, tag="encb")
            nc.sync.dma_start(out=encb[:, :], in_=ag3_out.to_broadcast([128, E]))

            # ======== episodes: norms (ACT) + dots (DVE mult, split reduce) ====
            dotA = small.tile([128, EPT], f32, tag="dotA")
            dotB = small.tile([128, EPT], f32, tag="dotB")
            nsq = small.tile([128, EPT], f32, tag="nsq")
            trash = trashp.tile([EPP, E], bf16, tag="trash")
            for t in range(EPT):
                et = eppool.tile([EPP, E], f32, tag="ep")
                ep_dma = nc.sync.dma_start(out=et[:, :], in_=epv[t])
                add_dep_helper(ep_dma.ins, ar1_write.ins,
                               reason="episode stream after E1 weight stream")
                trash2 = trash2p.tile([EPP, E], bf16, tag="trash2")
                nc.scalar.activation(out=trash[:, :], in_=et[:, :], func=AF.Square,
                                     accum_out=nsq[:EPP, t:t + 1])
                nc.vector.tensor_tensor(out=trash2[:, :], in0=et[:, :],
                                        in1=encb[:EPP, :], op=OP.mult)
                nc.scalar.activation(out=trash2[:, :EH], in_=trash2[:, :EH],
                                     func=AF.Copy, accum_out=dotA[:EPP, t:t + 1])
                nc.vector.tensor_reduce(out=dotB[:EPP, t:t + 1],
                                        in_=trash2[:, EH:],
                                        axis=mybir.AxisListType.X, op=OP.add)

            # ======== normalize + local top-k ========
            sraw = small.tile([128, EPT], f32, tag="sraw")
            nc.vector.tensor_add(out=sraw[:EPP, :], in0=dotA[:EPP, :], in1=dotB[:EPP, :])
            nstd = small.tile([128, EPT], f32, tag="nstd")
            nc.scalar.activation(out=nstd[:EPP, :], in_=nsq[:EPP, :], func=AF.Sqrt)
            nc.vector.reciprocal(out=nstd[:EPP, :], in_=nstd[:EPP, :])
            snorm = small.tile([128, EPT], f32, tag="snorm")
            nc.vector.tensor_mul(out=snorm[:EPP, :], in0=sraw[:EPP, :], in1=nstd[:EPP, :])
            nc.sync.dma_start(out=flat_d.rearrange("(p t) -> p t", t=EPT),
                              in_=snorm[:EPP, :])
            flat = small.tile([1, ES], f32, tag="flat")
            nc.sync.dma_start(out=flat[:1, :],
                              in_=flat_d.rearrange("(a n) -> a n", a=1))
            vals = small.tile([1, 8], f32, tag="vals")
            nc.vector.max(out=vals[:, :], in_=flat[:, :])
            idx8 = small.tile([1, 8], u32, tag="idx8")
            nc.vector.max_index(out=idx8[:, :], in_max=vals[:, :], in_values=flat[:, :])
            nc.sync.dma_start(out=idx_d.rearrange("(a n) -> a n", a=1),
                              in_=idx8[:, 0:K])
            idx3 = small.tile([K, 1], u32, tag="idx3")
            nc.sync.dma_start(out=idx3[:, :],
                              in_=idx_d.rearrange("(p o) -> p o", o=1))

            rows = small.tile([K, E], f32, tag="rows")
            nc.gpsimd.indirect_dma_start(
                out=rows[:, :], out_offset=None,
                in_=ep_s[:, :],
                in_offset=bass.IndirectOffsetOnAxis(ap=idx3[:, :1], axis=0),
            )

            # ======== decoder ========
            rowsT = small.tile([128, E // 128, K], wdt, tag="rowsT")
            for kc in range(E // 128):
                tp = psum_tp.tile([128, K], f32, tag="tp")
                nc.tensor.transpose(out=tp[:, :], in_=rows[:, 128 * kc:128 * (kc + 1)],
                                    identity=eye3sb[:, :])
                nc.vector.tensor_copy(out=rowsT[:, kc, :], in_=tp[:, :])
            pdp = psum.tile([K, H2], f32, tag="pdp")
            for kc in range(E // 128):
                wt = wd1p.tile([128, H2], wdt, tag="wd1")
                wd1_dma = nc.sync.dma_start(out=wt[:, :], in_=Wd1v[kc])
                add_dep_helper(wd1_dma.ins, ar1_write.ins,
                               reason="Wd1 stream after E1 weight stream")
                nc.tensor.matmul(
                    out=pdp[:, :], lhsT=rowsT[:, kc, :], rhs=wt[:, :],
                    start=(kc == 0), stop=(kc == E // 128 - 1),
                )
            d = small.tile([K, H2], f32, tag="d")
            nc.vector.tensor_copy(out=d[:, :], in_=pdp[:, :])
            nc.vector.tensor_add(out=d[:, :], in0=d[:, :], in1=bd1sb[:, :])
            nc.scalar.activation(out=d[:, :], in_=d[:, :], func=GELU)
            std = small.tile([K, 6], f32, tag="std")
            nc.vector.bn_stats(out=std[:, :], in_=d[:, :])
            mvd = small.tile([K, 2], f32, tag="mvd")
            nc.vector.bn_aggr(out=mvd[:, :], in_=std[:, :])
            rstdd = small.tile([K, 1], f32, tag="rstdd")
            nc.scalar.activation(out=rstdd[:, :], in_=mvd[:, 1:2], func=AF.Sqrt,
                                 bias=eps3[:, :])
            nc.vector.reciprocal(out=rstdd[:, :], in_=rstdd[:, :])
            nc.vector.tensor_scalar(
                out=d[:, :], in0=d[:, :],
                scalar1=mvd[:, 0:1], scalar2=rstdd[:, :],
                op0=OP.subtract, op1=OP.mult,
            )
            nc.vector.tensor_mul(out=d[:, :], in0=d[:, :], in1=gdsb[:, :])
            nc.vector.tensor_add(out=d[:, :], in0=d[:, :], in1=bedsb[:, :])

            dT = small.tile([128, C2, K], wdt, tag="dT")
            for kc in range(C2):
                tp = psum_tp.tile([128, K], f32, tag="tp")
                nc.tensor.transpose(out=tp[:, :], in_=d[:, 128 * kc:128 * (kc + 1)],
                                    identity=eye3sb[:, :])
                nc.vector.tensor_copy(out=dT[:, kc, :], in_=tp[:, :])
            o3p = psum.tile([K, DIM], f32, tag="o3p")
            for kc in range(C2):
                nc.tensor.matmul(
                    out=o3p[:, :], lhsT=dT[:, kc, :], rhs=Wd2sb[:, kc, :],
                    start=(kc == 0), stop=(kc == C2 - 1),
                )
            o3 = small.tile([K, DIM], f32, tag="o3")
            nc.vector.tensor_copy(out=o3[:, :], in_=o3p[:, :])
            nc.vector.tensor_add(out=o3[:, :], in0=o3[:, :], in1=bd2sb[:, :])

            nc.sync.dma_start(out=loc_out[:, :], in_=o3[:, :])
            nc.sync.dma_start(out=loc_sims[:, :], in_=vals[:, :])

    nc.compile()
    return nc


def _wcast(a):
    if not BF16:
        return np.ascontiguousarray(a, dtype=np.float32)
    import ml_dtypes
    return np.ascontiguousarray(np.asarray(a, dtype=np.float32).astype(ml_dtypes.bfloat16))


def _shard_inputs(buffer_states, episodes_encoded, W1, b1, g1, be1, W2, b2, g2,
                  be2, W3, b3, Wd1, bd1, gd, bed, Wd2, bd2):
    q = np.ascontiguousarray(buffer_states, dtype=np.float32).reshape(-1)
    eye3 = np.eye(3, dtype=np.float32)
    W2c = _wcast(W2)
    Wd1c = _wcast(Wd1)
    Wd2c = _wcast(Wd2)
    in_maps = []
    for i in range(NCORES):
        m = {
            "q_s": _wcast(q[QS * i:QS * (i + 1)]),
            "W1_s": _wcast(W1[QS * i:QS * (i + 1)]),
            "W2": W2c,
            "W3_s": _wcast(W3[:, ESLC * i:ESLC * (i + 1)]),
            "ep_s": np.ascontiguousarray(episodes_encoded[ES * i:ES * (i + 1)]),
            "Wd1": Wd1c,
            "Wd2": Wd2c,
            "b1v": b1, "g1v": g1, "be1v": be1,
            "b2v": b2, "g2v": g2, "be2v": be2,
            "b3s": np.ascontiguousarray(b3.reshape(1, -1)[:, ESLC * i:ESLC * (i + 1)]),
            "bd1v": bd1.reshape(1, -1), "gdv": gd.reshape(1, -1),
            "bedv": bed.reshape(1, -1), "bd2v": bd2.reshape(1, -1),
            "eye3": eye3,
        }
        in_maps.append(m)
    return in_maps


def _merge(results):
    sims24 = np.concatenate([r["loc_sims"][0, :K] for r in results])     # [24]
    outs24 = np.concatenate([r["loc_out"] for r in results], axis=0)     # [24, 256]
    top = np.argsort(-sims24, kind="stable")[:K]
    return outs24[top].mean(axis=0).astype(np.float32)


def kernel(*, trace=False, **inputs):
    global _compiled
    from concourse.bass_utils import run_bass_kernel_spmd

    k = int(inputs.pop("k"))
    assert k == K, f"kernel hardcodes k=3, got {k}"
    arrs = {name: np.ascontiguousarray(np.asarray(v, dtype=np.float32))
            for name, v in inputs.items()}
    in_maps = _shard_inputs(
        arrs["buffer_states"], arrs["episodes_encoded"],
        arrs["W1"], arrs["b1"], arrs["g1"], arrs["be1"],
        arrs["W2"], arrs["b2"], arrs["g2"], arrs["be2"],
        arrs["W3"], arrs["b3"], arrs["Wd1"], arrs["bd1"], arrs["gd"],
        arrs["bed"], arrs["Wd2"], arrs["bd2"],
    )
    if _compiled is None:
        _compiled = build_kernel()
    res = run_bass_kernel_spmd(_compiled, in_maps, core_ids=list(range(NCORES)),
                               trace=trace)
    out = _merge(res.results)
    if trace:
        kernel.last_exec_time_ns = res.exec_time_ns
    return out


kernel.last_exec_time_ns = None


# revision 18
# speedup vs baseline: 1.4983x; 1.0467x over previous
"""EpisodicMemory retrieval kernel for 8 Trainium2 NeuronCores.

Sharding (hardcoded for the nn_EpisodicMemory problem):
  - q = buffer_states.reshape(-1) [25600]: contraction-sharded for layer 1
    (each core gets q[3200i:3200(i+1)] and W1 rows [3200i:3200(i+1), :]),
    partial pre-activations summed with an on-device AllReduce (the only
    collective).
  - W2/W3 replicated in bf16; every core computes the full enc locally.
  - episodes_encoded row-sharded: core i scores episodes [1250i:1250(i+1)),
    computes local top-3, decodes them locally with a replicated Wd1/Wd2.
  - host merges the 8x3 candidates into the global top-3 and averages the
    matching decoded vectors (pure gather/selection glue).

Precision: weights are cast to bf16 on the host; episode data stays fp32 and
all matmuls accumulate in fp32 PSUM. The encoder only influences WHICH
episodes are selected (top-3 margins are ~10%), so this does not change the
selected set; the bf16 decoder weights give ~4e-3 relative output error.
Set BF16=False for a full-fp32 fallback.
"""

import numpy as np

DIM = 256
WIN = 100
COMP = 16
NEP = 10000
NCORES = 8

Q = WIN * DIM            # 25600
H1 = 4 * DIM             # 1024
H2 = 2 * DIM             # 512
E = COMP * DIM           # 4096
QS = Q // NCORES         # 3200 rows of W1 per core
ES = NEP // NCORES       # 1250 episodes per core
EPT = 10                 # episode tiles per core
EPP = ES // EPT          # 125 partitions used per episode tile
K = 3
EPS = 1e-5
BF16 = True
EP_BUFS = 5
EH = 2560                # ACT reduces cols [0:EH), DVE reduces [EH:E)

_compiled = {}


def build_kernel(gelu_func_name: str = "Gelu", zero_bias=False, unit_affine=False):
    import concourse.bacc as bacc
    import concourse.bass as bass
    import concourse.tile as tile
    import concourse.mybir as mybir
    from concourse.tile import add_dep_helper

    f32 = mybir.dt.float32
    u32 = mybir.dt.uint32
    bf16 = mybir.dt.bfloat16
    wdt = bf16 if BF16 else f32
    AF = mybir.ActivationFunctionType
    GELU = getattr(AF, gelu_func_name)
    OP = mybir.AluOpType

    nc = bacc.Bacc("TRN2", target_bir_lowering=False, debug=False,
                   enable_asserts=True, num_devices=NCORES)

    # ---- I/O ----
    q_s = nc.dram_tensor("q_s", [QS], wdt, kind="ExternalInput").ap()
    W1_s = nc.dram_tensor("W1_s", [QS, H1], wdt, kind="ExternalInput").ap()
    W2 = nc.dram_tensor("W2", [H1, H2], wdt, kind="ExternalInput").ap()
    W3 = nc.dram_tensor("W3", [H2, E], wdt, kind="ExternalInput").ap()
    ep_s = nc.dram_tensor("ep_s", [ES, E], f32, kind="ExternalInput").ap()
    Wd1 = nc.dram_tensor("Wd1", [E, H2], wdt, kind="ExternalInput").ap()
    Wd2 = nc.dram_tensor("Wd2", [H2, DIM], wdt, kind="ExternalInput").ap()
    vecs = {}
    if not zero_bias:
        for nm, width in [("b1v", H1), ("b2v", H2), ("b3v", E), ("bd1v", H2),
                          ("bd2v", DIM)]:
            vecs[nm] = nc.dram_tensor(nm, [width], f32, kind="ExternalInput").ap()
    if not unit_affine:
        for nm, width in [("g1v", H1), ("be1v", H1), ("g2v", H2), ("be2v", H2),
                          ("gdv", H2), ("bedv", H2)]:
            vecs[nm] = nc.dram_tensor(nm, [width], f32, kind="ExternalInput").ap()
    eye3 = nc.dram_tensor("eye3", [3, 3], f32, kind="ExternalInput").ap()

    loc_out = nc.dram_tensor("loc_out", [K, DIM], f32, kind="ExternalOutput").ap()
    loc_sims = nc.dram_tensor("loc_sims", [1, 8], f32, kind="ExternalOutput").ap()

    W1v = W1_s.rearrange("(kc p) n -> kc p n", p=128)          # [25,128,1024]
    W2v = W2.rearrange("(kc p) n -> kc p n", p=128)            # [8,128,512]
    W3v = W3.rearrange("(kc p) (cg n) -> cg kc p n", p=128, cg=4)  # [4,4,128,1024]
    epv = ep_s.rearrange("(p t) d -> t p d", t=EPT)            # [10,125,4096]
    Wd1v = Wd1.rearrange("(kc p) n -> kc p n", p=128)          # [32,128,512]

    C1 = H1 // 128   # 8
    C2 = H2 // 128   # 4

    with tile.TileContext(nc) as tc:
        with tc.tile_pool(name="dram", bufs=1, space="DRAM") as dram, \
             tc.tile_pool(name="const", bufs=1) as const, \
             tc.tile_pool(name="w1p", bufs=4) as w1p, \
             tc.tile_pool(name="encp", bufs=1) as encp, \
             tc.tile_pool(name="epp", bufs=EP_BUFS) as eppool, \
             tc.tile_pool(name="trash", bufs=1) as trashp, \
             tc.tile_pool(name="trash2", bufs=2) as trash2p, \
             tc.tile_pool(name="wd1p", bufs=4) as wd1p, \
             tc.tile_pool(name="small", bufs=1) as small, \
             tc.tile_pool(name="psum", bufs=2, space="PSUM") as psum, \
             tc.tile_pool(name="psum_tp", bufs=2, space="PSUM") as psum_tp:

            late_dmas = []

            def cvec(nm, width, tag):
                t = const.tile([1, width], f32, tag=tag)
                late_dmas.append(nc.sync.dma_start(
                    out=t[:, :], in_=vecs[nm].rearrange("(a n) -> a n", a=1)))
                return t

            def cvec_b(nm, width, tag):
                t = const.tile([K, width], f32, tag=tag)
                late_dmas.append(nc.sync.dma_start(
                    out=t[:, :],
                    in_=vecs[nm].rearrange("(a n) -> a n", a=1).to_broadcast([K, width])))
                return t

            # ---------- constants ----------
            qsb = const.tile([128, QS // 128], wdt, tag="qsb")
            nc.sync.dma_start(out=qsb[:, :], in_=q_s.rearrange("(kc p) -> p kc", p=128))
            Wd2sb = const.tile([128, C2, DIM], wdt, tag="wd2sb")
            late_dmas.append(nc.sync.dma_start(
                out=Wd2sb[:, :, :], in_=Wd2.rearrange("(kc p) n -> p kc n", p=128)))

            b1sb = cvec("b1v", H1, "b1sb") if not zero_bias else None
            b2sb = cvec("b2v", H2, "b2sb") if not zero_bias else None
            b3sb = cvec("b3v", E, "b3sb") if not zero_bias else None
            bd1sb = cvec_b("bd1v", H2, "bd1sb") if not zero_bias else None
            bd2sb = cvec_b("bd2v", DIM, "bd2sb") if not zero_bias else None
            g1sb = cvec("g1v", H1, "g1sb") if not unit_affine else None
            be1sb = cvec("be1v", H1, "be1sb") if not unit_affine else None
            g2sb = cvec("g2v", H2, "g2sb") if not unit_affine else None
            be2sb = cvec("be2v", H2, "be2sb") if not unit_affine else None
            gdsb = cvec_b("gdv", H2, "gdsb") if not unit_affine else None
            bedsb = cvec_b("bedv", H2, "bedsb") if not unit_affine else None

            eye3sb = const.tile([3, 3], f32, tag="eye3sb")
            late_dmas.append(nc.sync.dma_start(out=eye3sb[:, :], in_=eye3[:, :]))
            eps1 = const.tile([1, 1], f32, tag="eps1")
            nc.vector.memset(eps1[:, :], EPS)
            eps3 = const.tile([K, 1], f32, tag="eps3")
            nc.vector.memset(eps3[:, :], EPS)

            # DRAM bounce/scratch
            ar1_in = dram.tile([H1], f32)
            ar1_out = dram.tile([H1], f32)
            h1_d = dram.tile([H1], wdt)
            h2_d = dram.tile([H2], wdt)
            enc_d = dram.tile([E], f32)
            flat_d = dram.tile([ES], f32)
            idx_d = dram.tile([K], u32)

            # ======== E1: h1_pre = q_s @ W1_s  -> psum [1, 1024] ========
            e1p = psum.tile([1, H1], f32, tag="mm")
            nkc = QS // 128  # 25
            for kc in range(nkc):
                w1t = w1p.tile([128, H1], wdt, tag="w1")
                nc.sync.dma_start(out=w1t[:, :], in_=W1v[kc])
                for h in range(2):
                    nc.tensor.matmul(
                        out=e1p[:, 512 * h:512 * (h + 1)],
                        lhsT=qsb[:, kc:kc + 1],
                        rhs=w1t[:, 512 * h:512 * (h + 1)],
                        start=(kc == 0), stop=(kc == nkc - 1),
                    )
            h1f = small.tile([1, H1], f32, tag="h1flat")
            nc.vector.tensor_copy(out=h1f[:, :], in_=e1p[:, :])
            ar1_write = nc.sync.dma_start(out=ar1_in.rearrange("(a n) -> a n", a=1),
                                          in_=h1f[:, :])
            for _h in late_dmas:
                add_dep_helper(_h.ins, ar1_write.ins, reason="defer const loads")
            nc.gpsimd.collective_compute(
                "AllReduce", OP.add,
                replica_groups=[list(range(NCORES))],
                ins=[ar1_in.opt()], outs=[ar1_out.opt()],
            )

            def ln_flat(xf, width, bsb, gsb, besb, name):
                if bsb is not None:
                    nc.vector.tensor_add(out=xf[:, :], in0=xf[:, :], in1=bsb[:, :])
                nc.scalar.activation(out=xf[:, :], in_=xf[:, :], func=GELU)
                nsub = (width + 511) // 512
                st = small.tile([1, nsub, 6], f32, tag=f"st_{name}")
                for sg in range(nsub):
                    nc.vector.bn_stats(out=st[:, sg, :],
                                       in_=xf[:, 512 * sg:512 * (sg + 1)])
                mv = small.tile([1, 2], f32, tag=f"mv_{name}")
                nc.vector.bn_aggr(out=mv[:, :], in_=st[:, :, :])
                rstd = small.tile([1, 1], f32, tag=f"rstd_{name}")
                nc.scalar.activation(out=rstd[:, :], in_=mv[:, 1:2], func=AF.Sqrt,
                                     bias=eps1[:, :])
                nc.vector.reciprocal(out=rstd[:, :], in_=rstd[:, :])
                nc.vector.tensor_scalar(
                    out=xf[:, :], in0=xf[:, :],
                    scalar1=mv[:, 0:1], scalar2=rstd[:, :],
                    op0=OP.subtract, op1=OP.mult,
                )
                if gsb is not None:
                    nc.vector.tensor_mul(out=xf[:, :], in0=xf[:, :], in1=gsb[:, :])
                    nc.vector.tensor_add(out=xf[:, :], in0=xf[:, :], in1=besb[:, :])

            # ---------- E1 epilogue ----------
            h1 = small.tile([1, H1], f32, tag="h1flat")
            nc.sync.dma_start(out=h1[:, :], in_=ar1_out.rearrange("(a n) -> a n", a=1))
            ln_flat(h1, H1, b1sb, g1sb, be1sb, "l1")
            h1c = small.tile([1, H1], wdt, tag="h1c")
            nc.vector.tensor_copy(out=h1c[:, :], in_=h1[:, :])
            nc.sync.dma_start(out=h1_d.rearrange("(a n) -> a n", a=1), in_=h1c[:, :])
            h1m = small.tile([128, C1], wdt, tag="h1m")
            nc.sync.dma_start(out=h1m[:, :], in_=h1_d.rearrange("(kc p) -> p kc", p=128))

            # ======== E2 ========
            e23p = psum.tile([1, H2], f32, tag="mm")
            for kc in range(C1):
                w2t = w1p.tile([128, H2], wdt, tag="w1")
                nc.sync.dma_start(out=w2t[:, :], in_=W2v[kc])
                nc.tensor.matmul(
                    out=e23p[:, :], lhsT=h1m[:, kc:kc + 1], rhs=w2t[:, :],
                    start=(kc == 0), stop=(kc == C1 - 1),
                )
            h2 = small.tile([1, H2], f32, tag="h2flat")
            nc.vector.tensor_copy(out=h2[:, :], in_=e23p[:, :])
            ln_flat(h2, H2, b2sb, g2sb, be2sb, "l2")
            h2c = small.tile([1, H2], wdt, tag="h2c")
            nc.vector.tensor_copy(out=h2c[:, :], in_=h2[:, :])
            nc.sync.dma_start(out=h2_d.rearrange("(a n) -> a n", a=1), in_=h2c[:, :])
            h2m = small.tile([128, C2], wdt, tag="h2m")
            nc.sync.dma_start(out=h2m[:, :], in_=h2_d.rearrange("(kc p) -> p kc", p=128))

            # ======== E3: full enc = h2 @ W3 (replicated W3) ========
            encf = small.tile([1, E], f32, tag="encf")
            for cg in range(4):
                e3p = psum.tile([1, H1], f32, tag="mm")
                for kc in range(C2):
                    w3t = w1p.tile([128, H1], wdt, tag="w1")
                    nc.sync.dma_start(out=w3t[:, :], in_=W3v[cg, kc])
                    for h in range(2):
                        nc.tensor.matmul(
                            out=e3p[:, 512 * h:512 * (h + 1)],
                            lhsT=h2m[:, kc:kc + 1],
                            rhs=w3t[:, 512 * h:512 * (h + 1)],
                            start=(kc == 0), stop=(kc == C2 - 1),
                        )
                nc.vector.tensor_copy(out=encf[:, 1024 * cg:1024 * (cg + 1)], in_=e3p[:, :])
            if b3sb is not None:
                nc.vector.tensor_add(out=encf[:, :], in0=encf[:, :], in1=b3sb[:, :])
            nc.sync.dma_start(out=enc_d.rearrange("(a n) -> a n", a=1), in_=encf[:, :])
            encb = encp.tile([128, E], f32, tag="encb")
            nc.sync.dma_start(out=encb[:, :],
                              in_=enc_d.rearrange("(a n) -> a n", a=1).to_broadcast([128, E]))

            # ======== episodes ========
            dotA = small.tile([128, EPT], f32, tag="dotA")
            dotB = small.tile([128, EPT], f32, tag="dotB")
            nsq = small.tile([128, EPT], f32, tag="nsq")
            trash = trashp.tile([EPP, E], bf16, tag="trash")
            ep_dmas = []
            for t in range(EPT):
                et = eppool.tile([EPP, E], f32, tag="ep")
                ep_dma = nc.sync.dma_start(out=et[:, :], in_=epv[t])
                add_dep_helper(ep_dma.ins, ar1_write.ins,
                               reason="episode stream after E1 weight stream")
                ep_dmas.append(ep_dma)
                trash2 = trash2p.tile([EPP, E], bf16, tag="trash2")
                nc.scalar.activation(out=trash[:, :], in_=et[:, :], func=AF.Square,
                                     accum_out=nsq[:EPP, t:t + 1])
                nc.vector.tensor_tensor(out=trash2[:, :], in0=et[:, :],
                                        in1=encb[:EPP, :], op=OP.mult)
                nc.scalar.activation(out=trash2[:, :EH], in_=trash2[:, :EH],
                                     func=AF.Copy, accum_out=dotA[:EPP, t:t + 1])
                nc.vector.tensor_reduce(out=dotB[:EPP, t:t + 1],
                                        in_=trash2[:, EH:],
                                        axis=mybir.AxisListType.X, op=OP.add)

            # ======== normalize + local top-k ========
            sraw = small.tile([128, EPT], f32, tag="sraw")
            nc.vector.tensor_add(out=sraw[:EPP, :], in0=dotA[:EPP, :], in1=dotB[:EPP, :])
            nstd = small.tile([128, EPT], f32, tag="nstd")
            nc.scalar.activation(out=nstd[:EPP, :], in_=nsq[:EPP, :], func=AF.Sqrt)
            nc.vector.reciprocal(out=nstd[:EPP, :], in_=nstd[:EPP, :])
            snorm = small.tile([128, EPT], f32, tag="snorm")
            nc.vector.tensor_mul(out=snorm[:EPP, :], in0=sraw[:EPP, :], in1=nstd[:EPP, :])
            nc.sync.dma_start(out=flat_d.rearrange("(p t) -> p t", t=EPT),
                              in_=snorm[:EPP, :])
            flat = small.tile([1, ES], f32, tag="flat")
            nc.sync.dma_start(out=flat[:1, :],
                              in_=flat_d.rearrange("(a n) -> a n", a=1))
            vals = small.tile([1, 8], f32, tag="vals")
            nc.vector.max(out=vals[:, :], in_=flat[:, :])
            idx8 = small.tile([1, 8], u32, tag="idx8")
            nc.vector.max_index(out=idx8[:, :], in_max=vals[:, :], in_values=flat[:, :])
            nc.sync.dma_start(out=idx_d.rearrange("(a n) -> a n", a=1),
                              in_=idx8[:, 0:K])
            idx3 = small.tile([K, 1], u32, tag="idx3")
            nc.sync.dma_start(out=idx3[:, :],
                              in_=idx_d.rearrange("(p o) -> p o", o=1))

            rows = small.tile([K, E], f32, tag="rows")
            nc.gpsimd.indirect_dma_start(
                out=rows[:, :], out_offset=None,
                in_=ep_s[:, :],
                in_offset=bass.IndirectOffsetOnAxis(ap=idx3[:, :1], axis=0),
            )

            # ======== decoder ========
            rowsT = small.tile([128, E // 128, K], wdt, tag="rowsT")
            for kc in range(E // 128):
                tp = psum_tp.tile([128, K], f32, tag="tp")
                nc.tensor.transpose(out=tp[:, :], in_=rows[:, 128 * kc:128 * (kc + 1)],
                                    identity=eye3sb[:, :])
                nc.vector.tensor_copy(out=rowsT[:, kc, :], in_=tp[:, :])
            pdp = psum.tile([K, H2], f32, tag="mm")
            for kc in range(E // 128):
                wt = wd1p.tile([128, H2], wdt, tag="wd1")
                wd1_dma = nc.sync.dma_start(out=wt[:, :], in_=Wd1v[kc])
                add_dep_helper(wd1_dma.ins, ep_dmas[7].ins,
                               reason="Wd1 stream after bulk of episode stream")
                nc.tensor.matmul(
                    out=pdp[:, :], lhsT=rowsT[:, kc, :], rhs=wt[:, :],
                    start=(kc == 0), stop=(kc == E // 128 - 1),
                )
            d = small.tile([K, H2], f32, tag="d")
            nc.vector.tensor_copy(out=d[:, :], in_=pdp[:, :])
            if bd1sb is not None:
                nc.vector.tensor_add(out=d[:, :], in0=d[:, :], in1=bd1sb[:, :])
            nc.scalar.activation(out=d[:, :], in_=d[:, :], func=GELU)
            std = small.tile([K, 6], f32, tag="std")
            nc.vector.bn_stats(out=std[:, :], in_=d[:, :])
            mvd = small.tile([K, 2], f32, tag="mvd")
            nc.vector.bn_aggr(out=mvd[:, :], in_=std[:, :])
            rstdd = small.tile([K, 1], f32, tag="rstdd")
            nc.scalar.activation(out=rstdd[:, :], in_=mvd[:, 1:2], func=AF.Sqrt,
                                 bias=eps3[:, :])
            nc.vector.reciprocal(out=rstdd[:, :], in_=rstdd[:, :])
            nc.vector.tensor_scalar(
                out=d[:, :], in0=d[:, :],
                scalar1=mvd[:, 0:1], scalar2=rstdd[:, :],
                op0=OP.subtract, op1=OP.mult,
            )
            if gdsb is not None:
                nc.vector.tensor_mul(out=d[:, :], in0=d[:, :], in1=gdsb[:, :])
                nc.vector.tensor_add(out=d[:, :], in0=d[:, :], in1=bedsb[:, :])

            dT = small.tile([128, C2, K], wdt, tag="dT")
            for kc in range(C2):
                tp = psum_tp.tile([128, K], f32, tag="tp")
                nc.tensor.transpose(out=tp[:, :], in_=d[:, 128 * kc:128 * (kc + 1)],
                                    identity=eye3sb[:, :])
                nc.vector.tensor_copy(out=dT[:, kc, :], in_=tp[:, :])
            o3p = psum.tile([K, DIM], f32, tag="mm")
            for kc in range(C2):
                nc.tensor.matmul(
                    out=o3p[:, :], lhsT=dT[:, kc, :], rhs=Wd2sb[:, kc, :],
                    start=(kc == 0), stop=(kc == C2 - 1),
                )
            o3 = small.tile([K, DIM], f32, tag="o3")
            nc.vector.tensor_copy(out=o3[:, :], in_=o3p[:, :])
            if bd2sb is not None:
                nc.vector.tensor_add(out=o3[:, :], in0=o3[:, :], in1=bd2sb[:, :])

            nc.sync.dma_start(out=loc_out[:, :], in_=o3[:, :])
            nc.sync.dma_start(out=loc_sims[:, :], in_=vals[:, :])

    nc.compile()
    return nc


def _wcast(a):
    if not BF16:
        return np.ascontiguousarray(a, dtype=np.float32)
    import ml_dtypes
    return np.ascontiguousarray(np.asarray(a, dtype=np.float32).astype(ml_dtypes.bfloat16))


def _shard_inputs(buffer_states, episodes_encoded, W1, b1, g1, be1, W2, b2, g2,
                  be2, W3, b3, Wd1, bd1, gd, bed, Wd2, bd2, zero_bias, unit_affine):
    q = np.ascontiguousarray(buffer_states, dtype=np.float32).reshape(-1)
    eye3 = np.eye(3, dtype=np.float32)
    W2c = _wcast(W2)
    W3c = _wcast(W3)
    Wd1c = _wcast(Wd1)
    Wd2c = _wcast(Wd2)
    in_maps = []
    for i in range(NCORES):
        m = {
            "q_s": _wcast(q[QS * i:QS * (i + 1)]),
            "W1_s": _wcast(W1[QS * i:QS * (i + 1)]),
            "W2": W2c,
            "W3": W3c,
            "ep_s": np.ascontiguousarray(episodes_encoded[ES * i:ES * (i + 1)]),
            "Wd1": Wd1c,
            "Wd2": Wd2c,
            "eye3": eye3,
        }
        if not zero_bias:
            m.update({"b1v": b1, "b2v": b2, "b3v": b3, "bd1v": bd1, "bd2v": bd2})
        if not unit_affine:
            m.update({"g1v": g1, "be1v": be1, "g2v": g2, "be2v": be2,
                      "gdv": gd, "bedv": bed})
        in_maps.append(m)
    return in_maps


def _merge(results):
    sims24 = np.concatenate([r["loc_sims"][0, :K] for r in results])     # [24]
    outs24 = np.concatenate([r["loc_out"] for r in results], axis=0)     # [24, 256]
    top = np.argsort(-sims24, kind="stable")[:K]
    return outs24[top].mean(axis=0).astype(np.float32)


def kernel(*, trace=False, **inputs):
    from concourse.bass_utils import run_bass_kernel_spmd

    k = int(inputs.pop("k"))
    assert k == K, f"kernel hardcodes k=3, got {k}"
    arrs = {name: np.ascontiguousarray(np.asarray(v, dtype=np.float32))
            for name, v in inputs.items()}
    zero_bias = all(not arrs[n].any() for n in ("b1", "b2", "b3", "bd1", "bd2"))
    unit_affine = (all(np.all(arrs[n] == 1.0) for n in ("g1", "g2", "gd")) and
                   all(not arrs[n].any() for n in ("be1", "be2", "bed")))
    in_maps = _shard_inputs(
        arrs["buffer_states"], arrs["episodes_encoded"],
        arrs["W1"], arrs["b1"], arrs["g1"], arrs["be1"],
        arrs["W2"], arrs["b2"], arrs["g2"], arrs["be2"],
        arrs["W3"], arrs["b3"], arrs["Wd1"], arrs["bd1"], arrs["gd"],
        arrs["bed"], arrs["Wd2"], arrs["bd2"], zero_bias, unit_affine,
    )
    key = (zero_bias, unit_affine)
    if key not in _compiled:
        _compiled[key] = build_kernel(zero_bias=zero_bias, unit_affine=unit_affine)
    res = run_bass_kernel_spmd(_compiled[key], in_maps, core_ids=list(range(NCORES)),
                               trace=trace)
    out = _merge(res.results)
    if trace:
        kernel.last_exec_time_ns = res.exec_time_ns
    return out


kernel.last_exec_time_ns = None


# revision 19
# speedup vs baseline: 1.5178x; 1.0130x over previous
"""EpisodicMemory retrieval kernel for 8 Trainium2 NeuronCores.

Sharding (hardcoded for the nn_EpisodicMemory problem):
  - q = buffer_states.reshape(-1) [25600]: contraction-sharded for layer 1
    (each core gets q[3200i:3200(i+1)] and W1 rows [3200i:3200(i+1), :]),
    partial pre-activations summed with an on-device AllReduce (the only
    collective).
  - W2/W3 replicated in bf16; every core computes the full enc locally.
  - episodes_encoded row-sharded: core i scores episodes [1250i:1250(i+1)),
    computes local top-3, decodes them locally with a replicated Wd1/Wd2.
  - host merges the 8x3 candidates into the global top-3 and averages the
    matching decoded vectors (pure gather/selection glue).

Precision: weights are cast to bf16 on the host; episode data stays fp32 and
all matmuls accumulate in fp32 PSUM. The encoder only influences WHICH
episodes are selected (top-3 margins are ~10%), so this does not change the
selected set; the bf16 decoder weights give ~4e-3 relative output error.
Set BF16=False for a full-fp32 fallback.
"""

import numpy as np

DIM = 256
WIN = 100
COMP = 16
NEP = 10000
NCORES = 8

Q = WIN * DIM            # 25600
H1 = 4 * DIM             # 1024
H2 = 2 * DIM             # 512
E = COMP * DIM           # 4096
QS = Q // NCORES         # 3200 rows of W1 per core
ES = NEP // NCORES       # 1250 episodes per core
EPT = 10                 # episode tiles per core
EPP = ES // EPT          # 125 partitions used per episode tile
K = 3
EPS = 1e-5
BF16 = True
EP_BUFS = 6
EH = 2560                # ACT reduces cols [0:EH), DVE reduces [EH:E)

_compiled = {}


def build_kernel(gelu_func_name: str = "Gelu", zero_bias=False, unit_affine=False):
    import concourse.bacc as bacc
    import concourse.bass as bass
    import concourse.tile as tile
    import concourse.mybir as mybir
    from concourse.tile import add_dep_helper

    f32 = mybir.dt.float32
    u32 = mybir.dt.uint32
    bf16 = mybir.dt.bfloat16
    wdt = bf16 if BF16 else f32
    AF = mybir.ActivationFunctionType
    GELU = getattr(AF, gelu_func_name)
    OP = mybir.AluOpType

    nc = bacc.Bacc("TRN2", target_bir_lowering=False, debug=False,
                   enable_asserts=True, num_devices=NCORES)

    # ---- I/O ----
    q_s = nc.dram_tensor("q_s", [QS], wdt, kind="ExternalInput").ap()
    W1_s = nc.dram_tensor("W1_s", [QS, H1], wdt, kind="ExternalInput").ap()
    W2 = nc.dram_tensor("W2", [H1, H2], wdt, kind="ExternalInput").ap()
    W3 = nc.dram_tensor("W3", [H2, E], wdt, kind="ExternalInput").ap()
    ep_s = nc.dram_tensor("ep_s", [ES, E], f32, kind="ExternalInput").ap()
    Wd1 = nc.dram_tensor("Wd1", [E, H2], wdt, kind="ExternalInput").ap()
    Wd2 = nc.dram_tensor("Wd2", [H2, DIM], wdt, kind="ExternalInput").ap()
    vecs = {}
    if not zero_bias:
        for nm, width in [("b1v", H1), ("b2v", H2), ("b3v", E), ("bd1v", H2),
                          ("bd2v", DIM)]:
            vecs[nm] = nc.dram_tensor(nm, [width], f32, kind="ExternalInput").ap()
    if not unit_affine:
        for nm, width in [("g1v", H1), ("be1v", H1), ("g2v", H2), ("be2v", H2),
                          ("gdv", H2), ("bedv", H2)]:
            vecs[nm] = nc.dram_tensor(nm, [width], f32, kind="ExternalInput").ap()
    eye3 = nc.dram_tensor("eye3", [3, 3], f32, kind="ExternalInput").ap()

    loc_out = nc.dram_tensor("loc_out", [K, DIM], f32, kind="ExternalOutput").ap()
    loc_sims = nc.dram_tensor("loc_sims", [1, 8], f32, kind="ExternalOutput").ap()

    W1v = W1_s.rearrange("(kc p) n -> kc p n", p=128)          # [25,128,1024]
    W2v = W2.rearrange("(kc p) n -> kc p n", p=128)            # [8,128,512]
    W3v = W3.rearrange("(kc p) (cg n) -> cg kc p n", p=128, cg=4)  # [4,4,128,1024]
    epv = ep_s.rearrange("(p t) d -> t p d", t=EPT)            # [10,125,4096]
    Wd1v = Wd1.rearrange("(kc p) n -> kc p n", p=128)          # [32,128,512]

    C1 = H1 // 128   # 8
    C2 = H2 // 128   # 4

    with tile.TileContext(nc) as tc:
        with tc.tile_pool(name="dram", bufs=1, space="DRAM") as dram, \
             tc.tile_pool(name="const", bufs=1) as const, \
             tc.tile_pool(name="w1p", bufs=4) as w1p, \
             tc.tile_pool(name="encp", bufs=1) as encp, \
             tc.tile_pool(name="epp", bufs=EP_BUFS) as eppool, \
             tc.tile_pool(name="trash", bufs=1) as trashp, \
             tc.tile_pool(name="trash2", bufs=2) as trash2p, \
             tc.tile_pool(name="wd1p", bufs=4) as wd1p, \
             tc.tile_pool(name="small", bufs=1) as small, \
             tc.tile_pool(name="psum", bufs=2, space="PSUM") as psum, \
             tc.tile_pool(name="psum_tp", bufs=2, space="PSUM") as psum_tp:

            late_dmas = []

            def cvec(nm, width, tag):
                t = const.tile([1, width], f32, tag=tag)
                late_dmas.append(nc.sync.dma_start(
                    out=t[:, :], in_=vecs[nm].rearrange("(a n) -> a n", a=1)))
                return t

            def cvec_b(nm, width, tag):
                t = const.tile([K, width], f32, tag=tag)
                late_dmas.append(nc.sync.dma_start(
                    out=t[:, :],
                    in_=vecs[nm].rearrange("(a n) -> a n", a=1).to_broadcast([K, width])))
                return t

            # ---------- constants ----------
            qsb = const.tile([128, QS // 128], wdt, tag="qsb")
            nc.sync.dma_start(out=qsb[:, :], in_=q_s.rearrange("(kc p) -> p kc", p=128))
            Wd2sb = const.tile([128, C2, DIM], wdt, tag="wd2sb")
            late_dmas.append(nc.sync.dma_start(
                out=Wd2sb[:, :, :], in_=Wd2.rearrange("(kc p) n -> p kc n", p=128)))

            b1sb = cvec("b1v", H1, "b1sb") if not zero_bias else None
            b2sb = cvec("b2v", H2, "b2sb") if not zero_bias else None
            b3sb = cvec("b3v", E, "b3sb") if not zero_bias else None
            bd1sb = cvec_b("bd1v", H2, "bd1sb") if not zero_bias else None
            bd2sb = cvec_b("bd2v", DIM, "bd2sb") if not zero_bias else None
            g1sb = cvec("g1v", H1, "g1sb") if not unit_affine else None
            be1sb = cvec("be1v", H1, "be1sb") if not unit_affine else None
            g2sb = cvec("g2v", H2, "g2sb") if not unit_affine else None
            be2sb = cvec("be2v", H2, "be2sb") if not unit_affine else None
            gdsb = cvec_b("gdv", H2, "gdsb") if not unit_affine else None
            bedsb = cvec_b("bedv", H2, "bedsb") if not unit_affine else None

            eye3sb = const.tile([3, 3], f32, tag="eye3sb")
            late_dmas.append(nc.sync.dma_start(out=eye3sb[:, :], in_=eye3[:, :]))
            eps1 = const.tile([1, 1], f32, tag="eps1")
            nc.vector.memset(eps1[:, :], EPS)
            eps3 = const.tile([K, 1], f32, tag="eps3")
            nc.vector.memset(eps3[:, :], EPS)

            # DRAM bounce/scratch
            ar1_in = dram.tile([H1], f32)
            ar1_out = dram.tile([H1], f32)
            h1_d = dram.tile([H1], wdt)
            h2_d = dram.tile([H2], wdt)
            enc_d = dram.tile([E], f32)
            flat_d = dram.tile([ES], f32)
            idx_d = dram.tile([K], u32)

            # ======== E1: h1_pre = q_s @ W1_s  -> psum [1, 1024] ========
            e1p = psum.tile([1, H1], f32, tag="mm")
            nkc = QS // 128  # 25
            for kc in range(nkc):
                w1t = w1p.tile([128, H1], wdt, tag="w1")
                nc.sync.dma_start(out=w1t[:, :], in_=W1v[kc])
                for h in range(2):
                    nc.tensor.matmul(
                        out=e1p[:, 512 * h:512 * (h + 1)],
                        lhsT=qsb[:, kc:kc + 1],
                        rhs=w1t[:, 512 * h:512 * (h + 1)],
                        start=(kc == 0), stop=(kc == nkc - 1),
                    )
            h1f = small.tile([1, H1], f32, tag="h1flat")
            nc.vector.tensor_copy(out=h1f[:, :], in_=e1p[:, :])
            ar1_write = nc.sync.dma_start(out=ar1_in.rearrange("(a n) -> a n", a=1),
                                          in_=h1f[:, :])
            for _h in late_dmas:
                add_dep_helper(_h.ins, ar1_write.ins, reason="defer const loads")
            nc.gpsimd.collective_compute(
                "AllReduce", OP.add,
                replica_groups=[list(range(NCORES))],
                ins=[ar1_in.opt()], outs=[ar1_out.opt()],
            )

            def ln_flat(xf, xout, width, bsb, gsb, besb, name):
                """gelu+LN on [1,width] f32 xf; final normalized result -> xout."""
                if bsb is not None:
                    nc.vector.tensor_add(out=xf[:, :], in0=xf[:, :], in1=bsb[:, :])
                nc.scalar.activation(out=xf[:, :], in_=xf[:, :], func=GELU)
                nsub = (width + 511) // 512
                st = small.tile([1, nsub, 6], f32, tag=f"st_{name}")
                for sg in range(nsub):
                    nc.vector.bn_stats(out=st[:, sg, :],
                                       in_=xf[:, 512 * sg:512 * (sg + 1)])
                mv = small.tile([1, 2], f32, tag=f"mv_{name}")
                nc.vector.bn_aggr(out=mv[:, :], in_=st[:, :, :])
                rstd = small.tile([1, 1], f32, tag=f"rstd_{name}")
                nc.scalar.activation(out=rstd[:, :], in_=mv[:, 1:2], func=AF.Sqrt,
                                     bias=eps1[:, :])
                nc.vector.reciprocal(out=rstd[:, :], in_=rstd[:, :])
                last = xout if gsb is None else xf
                nc.vector.tensor_scalar(
                    out=last[:, :], in0=xf[:, :],
                    scalar1=mv[:, 0:1], scalar2=rstd[:, :],
                    op0=OP.subtract, op1=OP.mult,
                )
                if gsb is not None:
                    nc.vector.tensor_mul(out=xf[:, :], in0=xf[:, :], in1=gsb[:, :])
                    nc.vector.tensor_add(out=xout[:, :], in0=xf[:, :], in1=besb[:, :])

            # ---------- E1 epilogue ----------
            h1 = small.tile([1, H1], f32, tag="h1flat")
            nc.sync.dma_start(out=h1[:, :], in_=ar1_out.rearrange("(a n) -> a n", a=1))
            h1c = small.tile([1, H1], wdt, tag="h1c")
            ln_flat(h1, h1c, H1, b1sb, g1sb, be1sb, "l1")
            nc.sync.dma_start(out=h1_d.rearrange("(a n) -> a n", a=1), in_=h1c[:, :])
            h1m = small.tile([128, C1], wdt, tag="h1m")
            nc.sync.dma_start(out=h1m[:, :], in_=h1_d.rearrange("(kc p) -> p kc", p=128))

            # ======== E2 ========
            e23p = psum.tile([1, H2], f32, tag="mm")
            for kc in range(C1):
                w2t = w1p.tile([128, H2], wdt, tag="w1")
                nc.sync.dma_start(out=w2t[:, :], in_=W2v[kc])
                nc.tensor.matmul(
                    out=e23p[:, :], lhsT=h1m[:, kc:kc + 1], rhs=w2t[:, :],
                    start=(kc == 0), stop=(kc == C1 - 1),
                )
            h2 = small.tile([1, H2], f32, tag="h2flat")
            nc.vector.tensor_copy(out=h2[:, :], in_=e23p[:, :])
            h2c = small.tile([1, H2], wdt, tag="h2c")
            ln_flat(h2, h2c, H2, b2sb, g2sb, be2sb, "l2")
            nc.sync.dma_start(out=h2_d.rearrange("(a n) -> a n", a=1), in_=h2c[:, :])
            h2m = small.tile([128, C2], wdt, tag="h2m")
            nc.sync.dma_start(out=h2m[:, :], in_=h2_d.rearrange("(kc p) -> p kc", p=128))

            # ======== E3: full enc = h2 @ W3 (replicated W3) ========
            encf = small.tile([1, E], f32, tag="big16")
            for cg in range(4):
                e3p = psum.tile([1, H1], f32, tag="mm")
                for kc in range(C2):
                    w3t = w1p.tile([128, H1], wdt, tag="w1")
                    nc.sync.dma_start(out=w3t[:, :], in_=W3v[cg, kc])
                    for h in range(2):
                        nc.tensor.matmul(
                            out=e3p[:, 512 * h:512 * (h + 1)],
                            lhsT=h2m[:, kc:kc + 1],
                            rhs=w3t[:, 512 * h:512 * (h + 1)],
                            start=(kc == 0), stop=(kc == C2 - 1),
                        )
                nc.vector.tensor_copy(out=encf[:, 1024 * cg:1024 * (cg + 1)], in_=e3p[:, :])
            if b3sb is not None:
                nc.vector.tensor_add(out=encf[:, :], in0=encf[:, :], in1=b3sb[:, :])
            nc.sync.dma_start(out=enc_d.rearrange("(a n) -> a n", a=1), in_=encf[:, :])
            encb = encp.tile([128, E], f32, tag="encb")
            nc.sync.dma_start(out=encb[:, :],
                              in_=enc_d.rearrange("(a n) -> a n", a=1).to_broadcast([128, E]))

            # ======== episodes ========
            dotA = small.tile([128, EPT], f32, tag="dotA")
            dotB = small.tile([128, EPT], f32, tag="dotB")
            nsq = small.tile([128, EPT], f32, tag="nsq")
            trash = trashp.tile([EPP, E], bf16, tag="trash")
            ep_dmas = []
            for t in range(EPT):
                et = eppool.tile([EPP, E], f32, tag="ep")
                for hh in range(2):
                    ep_dma = nc.sync.dma_start(out=et[:, 2048 * hh:2048 * (hh + 1)],
                                               in_=epv[t][:, 2048 * hh:2048 * (hh + 1)])
                    add_dep_helper(ep_dma.ins, ar1_write.ins,
                                   reason="episode stream after E1 weight stream")
                    ep_dmas.append(ep_dma)
                trash2 = trash2p.tile([EPP, E], bf16, tag="trash2")
                nc.scalar.activation(out=trash[:, :], in_=et[:, :], func=AF.Square,
                                     accum_out=nsq[:EPP, t:t + 1])
                nc.vector.tensor_tensor(out=trash2[:, :], in0=et[:, :],
                                        in1=encb[:EPP, :], op=OP.mult)
                nc.scalar.activation(out=trash2[:, :EH], in_=trash2[:, :EH],
                                     func=AF.Copy, accum_out=dotA[:EPP, t:t + 1])
                nc.vector.tensor_reduce(out=dotB[:EPP, t:t + 1],
                                        in_=trash2[:, EH:],
                                        axis=mybir.AxisListType.X, op=OP.add)

            # ======== normalize + local top-k ========
            sraw = small.tile([128, EPT], f32, tag="sraw")
            nc.vector.tensor_add(out=sraw[:EPP, :], in0=dotA[:EPP, :], in1=dotB[:EPP, :])
            nstd = small.tile([128, EPT], f32, tag="nstd")
            nc.scalar.activation(out=nstd[:EPP, :], in_=nsq[:EPP, :], func=AF.Sqrt)
            nc.vector.reciprocal(out=nstd[:EPP, :], in_=nstd[:EPP, :])
            snorm = small.tile([128, EPT], f32, tag="snorm")
            nc.vector.tensor_mul(out=snorm[:EPP, :], in0=sraw[:EPP, :], in1=nstd[:EPP, :])
            nc.sync.dma_start(out=flat_d.rearrange("(p t) -> p t", t=EPT),
                              in_=snorm[:EPP, :])
            flat = small.tile([1, ES], f32, tag="flat")
            nc.sync.dma_start(out=flat[:1, :],
                              in_=flat_d.rearrange("(a n) -> a n", a=1))
            vals = small.tile([1, 8], f32, tag="vals")
            nc.vector.max(out=vals[:, :], in_=flat[:, :])
            idx8 = small.tile([1, 8], u32, tag="idx8")
            nc.vector.max_index(out=idx8[:, :], in_max=vals[:, :], in_values=flat[:, :])
            nc.sync.dma_start(out=idx_d.rearrange("(a n) -> a n", a=1),
                              in_=idx8[:, 0:K])
            idx3 = small.tile([K, 1], u32, tag="idx3")
            nc.sync.dma_start(out=idx3[:, :],
                              in_=idx_d.rearrange("(p o) -> p o", o=1))

            rows = small.tile([K, E], f32, tag="big16")
            nc.gpsimd.indirect_dma_start(
                out=rows[:, :], out_offset=None,
                in_=ep_s[:, :],
                in_offset=bass.IndirectOffsetOnAxis(ap=idx3[:, :1], axis=0),
            )

            # ======== decoder ========
            rowsT = small.tile([128, E // 128, K], wdt, tag="rowsT")
            pdp = psum.tile([K, H2], f32, tag="mm")
            for kc in range(E // 128):
                tp = psum_tp.tile([128, K], f32, tag="tp")
                nc.tensor.transpose(out=tp[:, :], in_=rows[:, 128 * kc:128 * (kc + 1)],
                                    identity=eye3sb[:, :])
                nc.vector.tensor_copy(out=rowsT[:, kc, :], in_=tp[:, :])
                wt = wd1p.tile([128, H2], wdt, tag="wd1")
                wd1_dma = nc.sync.dma_start(out=wt[:, :], in_=Wd1v[kc])
                add_dep_helper(wd1_dma.ins, ep_dmas[15].ins,
                               reason="Wd1 stream after bulk of episode stream")
                nc.tensor.matmul(
                    out=pdp[:, :], lhsT=rowsT[:, kc, :], rhs=wt[:, :],
                    start=(kc == 0), stop=(kc == E // 128 - 1),
                )
            d = small.tile([K, H2], f32, tag="d")
            nc.vector.tensor_copy(out=d[:, :], in_=pdp[:, :])
            if bd1sb is not None:
                nc.vector.tensor_add(out=d[:, :], in0=d[:, :], in1=bd1sb[:, :])
            nc.scalar.activation(out=d[:, :], in_=d[:, :], func=GELU)
            std = small.tile([K, 6], f32, tag="std")
            nc.vector.bn_stats(out=std[:, :], in_=d[:, :])
            mvd = small.tile([K, 2], f32, tag="mvd")
            nc.vector.bn_aggr(out=mvd[:, :], in_=std[:, :])
            rstdd = small.tile([K, 1], f32, tag="rstdd")
            nc.scalar.activation(out=rstdd[:, :], in_=mvd[:, 1:2], func=AF.Sqrt,
                                 bias=eps3[:, :])
            nc.vector.reciprocal(out=rstdd[:, :], in_=rstdd[:, :])
            nc.vector.tensor_scalar(
                out=d[:, :], in0=d[:, :],
                scalar1=mvd[:, 0:1], scalar2=rstdd[:, :],
                op0=OP.subtract, op1=OP.mult,
            )
            if gdsb is not None:
                nc.vector.tensor_mul(out=d[:, :], in0=d[:, :], in1=gdsb[:, :])
                nc.vector.tensor_add(out=d[:, :], in0=d[:, :], in1=bedsb[:, :])

            dT = small.tile([128, C2, K], wdt, tag="dT")
            for kc in range(C2):
                tp = psum_tp.tile([128, K], f32, tag="tp")
                nc.tensor.transpose(out=tp[:, :], in_=d[:, 128 * kc:128 * (kc + 1)],
                                    identity=eye3sb[:, :])
                nc.vector.tensor_copy(out=dT[:, kc, :], in_=tp[:, :])
            o3p = psum.tile([K, DIM], f32, tag="mm")
            for kc in range(C2):
                nc.tensor.matmul(
                    out=o3p[:, :], lhsT=dT[:, kc, :], rhs=Wd2sb[:, kc, :],
                    start=(kc == 0), stop=(kc == C2 - 1),
                )
            o3 = small.tile([K, DIM], f32, tag="o3")
            nc.vector.tensor_copy(out=o3[:, :], in_=o3p[:, :])
            if bd2sb is not None:
                nc.vector.tensor_add(out=o3[:, :], in0=o3[:, :], in1=bd2sb[:, :])

            nc.sync.dma_start(out=loc_out[:, :], in_=o3[:, :])
            nc.sync.dma_start(out=loc_sims[:, :], in_=vals[:, :])

    nc.compile()
    return nc


def _wcast(a):
    if not BF16:
        return np.ascontiguousarray(a, dtype=np.float32)
    import ml_dtypes
    return np.ascontiguousarray(np.asarray(a, dtype=np.float32).astype(ml_dtypes.bfloat16))


def _shard_inputs(buffer_states, episodes_encoded, W1, b1, g1, be1, W2, b2, g2,
                  be2, W3, b3, Wd1, bd1, gd, bed, Wd2, bd2, zero_bias, unit_affine):
    q = np.ascontiguousarray(buffer_states, dtype=np.float32).reshape(-1)
    eye3 = np.eye(3, dtype=np.float32)
    W2c = _wcast(W2)
    W3c = _wcast(W3)
    Wd1c = _wcast(Wd1)
    Wd2c = _wcast(Wd2)
    in_maps = []
    for i in range(NCORES):
        m = {
            "q_s": _wcast(q[QS * i:QS * (i + 1)]),
            "W1_s": _wcast(W1[QS * i:QS * (i + 1)]),
            "W2": W2c,
            "W3": W3c,
            "ep_s": np.ascontiguousarray(episodes_encoded[ES * i:ES * (i + 1)]),
            "Wd1": Wd1c,
            "Wd2": Wd2c,
            "eye3": eye3,
        }
        if not zero_bias:
            m.update({"b1v": b1, "b2v": b2, "b3v": b3, "bd1v": bd1, "bd2v": bd2})
        if not unit_affine:
            m.update({"g1v": g1, "be1v": be1, "g2v": g2, "be2v": be2,
                      "gdv": gd, "bedv": bed})
        in_maps.append(m)
    return in_maps


def _merge(results):
    sims24 = np.concatenate([r["loc_sims"][0, :K] for r in results])     # [24]
    outs24 = np.concatenate([r["loc_out"] for r in results], axis=0)     # [24, 256]
    top = np.argsort(-sims24, kind="stable")[:K]
    return outs24[top].mean(axis=0).astype(np.float32)


def kernel(*, trace=False, **inputs):
    from concourse.bass_utils import run_bass_kernel_spmd

    k = int(inputs.pop("k"))
    assert k == K, f"kernel hardcodes k=3, got {k}"
    arrs = {name: np.ascontiguousarray(np.asarray(v, dtype=np.float32))
            for name, v in inputs.items()}
    zero_bias = all(not arrs[n].any() for n in ("b1", "b2", "b3", "bd1", "bd2"))
    unit_affine = (all(np.all(arrs[n] == 1.0) for n in ("g1", "g2", "gd")) and
                   all(not arrs[n].any() for n in ("be1", "be2", "bed")))
    in_maps = _shard_inputs(
        arrs["buffer_states"], arrs["episodes_encoded"],
        arrs["W1"], arrs["b1"], arrs["g1"], arrs["be1"],
        arrs["W2"], arrs["b2"], arrs["g2"], arrs["be2"],
        arrs["W3"], arrs["b3"], arrs["Wd1"], arrs["bd1"], arrs["gd"],
        arrs["bed"], arrs["Wd2"], arrs["bd2"], zero_bias, unit_affine,
    )
    key = (zero_bias, unit_affine)
    if key not in _compiled:
        _compiled[key] = build_kernel(zero_bias=zero_bias, unit_affine=unit_affine)
    res = run_bass_kernel_spmd(_compiled[key], in_maps, core_ids=list(range(NCORES)),
                               trace=trace)
    out = _merge(res.results)
    if trace:
        kernel.last_exec_time_ns = res.exec_time_ns
    return out


kernel.last_exec_time_ns = None
